# revision 21
# baseline (speedup 1.0000x reference)
# Trainium2 Bass kernel for nn_Critic (RSA block critic over ragged agent sets).
#
# Strategy v2:
#  - Data-parallel over batch: 64 samples -> 8 cores x 8 samples, globally
#    sorted by length and snake-striped so all cores share ONE program.
#  - Activations feature-major ([feature, token]) bf16; fp32 PSUM accum.
#  - q/k keep their natural "4 heads per 32-partition band" layout straight
#    out of the QK projection; score matmuls are row-tiled (tile_position=
#    (32p, 0)) so 4 heads run concurrently in the PE array.
#  - All phase-B matmuls are single-shot (start&stop) PSUM groups; k-tile
#    accumulation for the den/ctx of two-tile slots happens on the DVE.
#    This avoids serialized per-bank accumulation chains.
#  - Few, large DMAs (one bf16 weight blob, column-split xT) on HWDGE
#    queues only (sync/scalar/vector); gpsimd does no DMA.
#  - ScalarE runs exp (big 1024-col instructions) + relu only; exp table
#    prefetched by a dummy exp at kernel start.
import math
import os

import numpy as np
import ml_dtypes

import concourse.bass as bass
import concourse.mybir as mybir
import concourse.tile as tile
from concourse import bacc
from concourse.bass_utils import run_bass_kernel_spmd

B, N, D, E, H, DH = 64, 256, 256, 256, 8, 32
NCORES, SPC = 8, 8
NEG = -1e9
PADW = 32
SCALE = 1.0 / math.sqrt(DH)
BF16 = ml_dtypes.bfloat16
AF = mybir.ActivationFunctionType
OP = mybir.AluOpType

# weight blob column offsets (bf16, [128, WBC])
WIN0, WQK0, WV0, WO0, WU0, WF0 = 0, 512, 1536, 2048, 2560, 3072
WBC = 3074

LAST_RESULT = None  # BassKernelResults of the most recent run (for test harness)


# ---------------------------------------------------------------- planning
def _plan(actives):
    a = np.asarray(actives).reshape(-1).astype(np.int64)
    assert a.shape == (B,)
    order = np.argsort(-a, kind="stable")
    slots = [[] for _ in range(NCORES)]
    for r, s in enumerate(order):
        stripe, pos = divmod(r, NCORES)
        c = pos if stripe % 2 == 0 else NCORES - 1 - pos
        slots[c].append(int(s))
    for c in range(NCORES):
        slots[c].sort(key=lambda s: -int(a[s]))
    ws = []
    for i in range(SPC):
        wi = max(int(a[slots[c][i]]) for c in range(NCORES))
        wi = max(PADW, ((wi + PADW - 1) // PADW) * PADW)
        ws.append(wi)
    kts = [(w + 127) // 128 for w in ws]
    offs = np.concatenate([[0], np.cumsum(ws)]).astype(int)
    kb = np.concatenate([[0], np.cumsum(kts)]).astype(int)
    return dict(
        a=a, slots=slots, ws=tuple(ws), kts=tuple(kts),
        offs=tuple(int(x) for x in offs[:-1]), T=int(offs[-1]),
        kb=tuple(int(x) for x in kb[:-1]), NKT=int(kb[-1]),
    )


# ---------------------------------------------------------------- program
_PROG_CACHE = {}


def _build_program(key):
    (T, ws, has_vbias) = key
    kts = tuple((w + 127) // 128 for w in ws)
    offs, kb = [], []
    o = k = 0
    for w, kt in zip(ws, kts):
        offs.append(o); kb.append(k); o += w; k += kt
    NKT = k
    BB = 8 + NKT + 1  # b_qk[0:4] b_oo[4:8] maskb[8:8+NKT] b_v[BB-1]
    dtb, dtf = mybir.dt.bfloat16, mybir.dt.float32
    cgroups = [(0, 1), (2, 3), (4, 5), (6, 7)]

    nc = bacc.Bacc("TRN2", target_bir_lowering=False, debug=False,
                   enable_asserts=False, num_devices=NCORES)

    def din(name, shape, dt):
        return nc.dram_tensor(name, shape, dt, kind="ExternalInput").ap()

    xT_d = din("xT", [258, T], dtb)
    wb_d = din("wb", [128, WBC], dtb)
    wr_d = din("wr", [2, 256], dtb)
    bb_d = din("bb", [128, BB], dtf)
    m01_d = din("mask01", [1, T], dtf)
    wvb_d = din("w_vb", [1, 256], dtb) if has_vbias else None
    out_d = nc.dram_tensor("val_out", [1, SPC], dtf, kind="ExternalOutput").ap()

    spl = min(T, 512)  # first xT column split (covers phase-A chunk 0)

    with tile.TileContext(nc) as tc:
        with (
            tc.tile_pool(name="const", bufs=1) as cp,
            tc.tile_pool(name="big", bufs=1) as bp,
            tc.tile_pool(name="vp", bufs=NKT) as vp,
            tc.tile_pool(name="ep", bufs=4) as ep,
            tc.tile_pool(name="rp", bufs=4) as rp,
            tc.tile_pool(name="pmm", bufs=2, space="PSUM") as pmm,
            tc.tile_pool(name="psc", bufs=2, space="PSUM") as psc,
            tc.tile_pool(name="pd", bufs=1, space="PSUM") as pd,
            tc.tile_pool(name="pc", bufs=1, space="PSUM") as pc,
        ):
            # ---- SBUF tiles
            wb_sb = cp.tile([128, WBC], dtb, tag="wb", name="wb")
            wr_sb = cp.tile([2, 256], dtb, tag="wr", name="wr")
            bb_sb = cp.tile([128, BB], dtf, tag="bb", name="bb")
            m01_sb = cp.tile([1, T], dtf, tag="m01", name="m01")
            ones_sb = cp.tile([128, 512], dtb, tag="ones", name="ones")
            dscr = cp.tile([1, 2], dtf, tag="dscr", name="dscr")
            xT_sb = [bp.tile([128, T], dtb, tag="xT0", name="xT0"),
                     bp.tile([128, T], dtb, tag="xT1", name="xT1"),
                     bp.tile([2, T], dtb, tag="xT2", name="xT2")]
            hT_sb = [bp.tile([128, T], dtb, tag=f"hT{f}", name=f"hT{f}")
                     for f in range(2)]
            q4_sb = [bp.tile([128, T], dtb, tag=f"q4{g}", name=f"q4{g}")
                     for g in range(2)]
            k4_sb = [bp.tile([128, T], dtb, tag=f"k4{g}", name=f"k4{g}")
                     for g in range(2)]
            # zero-padded per-head q: slot p holds head 4g+p's rows in band
            # 32p, zeros elsewhere -> K=128 score matmuls pick out one head
            # while sharing the k4 stationary (no base-0 relayout needed).
            qz_sb = [bp.tile([128, 4, T], dtb, tag=f"qz{g}", name=f"qz{g}")
                     for g in range(2)]
            ctxT_sb = [bp.tile([128, T], dtb, tag=f"cx{f}", name=f"cx{f}")
                       for f in range(2)]
            rsap_sb = [bp.tile([128, T], dtb, tag=f"rp{f}", name=f"rp{f}")
                       for f in range(2)]
            rsa_sb = [bp.tile([128, T], dtb, tag=f"rs{f}", name=f"rs{f}")
                      for f in range(2)]
            val_sb = bp.tile([1, T], dtf, tag="val", name="val")
            vscr_sb = bp.tile([1, 512], dtf, tag="vscr", name="vscr")
            out_sb = bp.tile([1, SPC], dtf, tag="out", name="out")
            if has_vbias:
                ones1_sb = cp.tile([1, T], dtb, tag="ones1", name="ones1")
                wvb_sb = cp.tile([1, 256], dtb, tag="wvb", name="wvb")

            # ---- DMA triggers (HWDGE only; order = need order)
            nc.scalar.dma_start(out=wb_sb[:, 0:WQK0], in_=wb_d[:, 0:WQK0])
            nc.scalar.dma_start(out=wb_sb[:, WQK0:WV0], in_=wb_d[:, WQK0:WV0])
            nc.gpsimd.dma_start(out=wr_sb, in_=wr_d)
            nc.gpsimd.dma_start(out=xT_sb[2], in_=xT_d[256:258, :])
            nc.sync.dma_start(out=xT_sb[0][:, 0:spl], in_=xT_d[0:128, 0:spl])
            nc.sync.dma_start(out=xT_sb[1][:, 0:spl], in_=xT_d[128:256, 0:spl])
            nc.sync.dma_start(out=bb_sb, in_=bb_d)
            if spl < T:
                nc.sync.dma_start(out=xT_sb[0][:, spl:T], in_=xT_d[0:128, spl:T])
                nc.sync.dma_start(out=xT_sb[1][:, spl:T], in_=xT_d[128:256, spl:T])
            nc.sync.dma_start(out=wb_sb[:, WV0:WBC], in_=wb_d[:, WV0:WBC])
            nc.sync.dma_start(out=m01_sb, in_=m01_d)
            if has_vbias:
                nc.sync.dma_start(out=wvb_sb, in_=wvb_d)
                nc.vector.memset(ones1_sb, 1.0)

            # exp table prefetch + constants
            nc.vector.memset(ones_sb, 1.0)
            nc.vector.memset(dscr[0:1, 0:1], 0.0)
            nc.scalar.activation(dscr[0:1, 1:2], dscr[0:1, 0:1], AF.Exp)

            mm = nc.tensor.matmul

            # PE warm-up: dense dummy matmuls on the ones tile while the
            # input DMAs stream, so HAM un-throttles before real work starts.
            wps = pmm.tile([128, 512], dtf, tag="mm", name="mm")
            for wi in range(6):
                mm(wps[0:64, 0:512], ones_sb[:, 0:64], ones_sb[:, 0:512],
                   start=True, stop=True)

            # ---------------- emission helpers
            def A_chunk(c0):
                cw = min(512, T - c0)
                cs = slice(c0, c0 + cw)
                for ft in range(2):
                    fs = slice(128 * ft, 128 * ft + 128)
                    hps = pmm.tile([128, 512], dtf, tag="mm", name="mm")
                    mm(hps[:, :cw], wb_sb[:, WIN0 + 128 * ft:WIN0 + 128 * ft + 128],
                       xT_sb[0][:, cs], start=True, stop=False)
                    mm(hps[:, :cw], wb_sb[:, WIN0 + 256 + 128 * ft:WIN0 + 384 + 128 * ft],
                       xT_sb[1][:, cs], start=False, stop=False)
                    mm(hps[:, :cw], wr_sb[:, fs], xT_sb[2][:, cs],
                       start=False, stop=True)
                    nc.scalar.activation(hT_sb[ft][:, cs], hps[:, :cw], AF.Relu)
                for m in range(4):
                    qps = pmm.tile([128, 512], dtf, tag="mm", name="mm")
                    mm(qps[:, :cw], wb_sb[:, WQK0 + 128 * m:WQK0 + 128 * m + 128],
                       hT_sb[0][:, cs], start=True, stop=False)
                    mm(qps[:, :cw], wb_sb[:, WQK0 + 512 + 128 * m:WQK0 + 640 + 128 * m],
                       hT_sb[1][:, cs], start=False, stop=True)
                    dst = (q4_sb if m < 2 else k4_sb)[m % 2]
                    nc.vector.tensor_scalar_add(dst[:, cs], qps[:, :cw],
                                                bb_sb[:, m:m + 1])

            vts = {}  # (slot, jj) -> bf16 v tile [nkz, 256]

            def V_slot(i):
                w, off = ws[i], offs[i]
                for jj in range(kts[i]):
                    nkz = min(128, w - 128 * jj)
                    t0 = off + 128 * jj
                    vps = pmm.tile([128, 256], dtf, tag="mm", name="mm")
                    mm(vps[0:nkz, :], hT_sb[0][:, t0:t0 + nkz],
                       wb_sb[:, WV0:WV0 + 256], start=True, stop=False)
                    mm(vps[0:nkz, :], hT_sb[1][:, t0:t0 + nkz],
                       wb_sb[:, WV0 + 256:WV0 + 512], start=False,
                       stop=not has_vbias)
                    if has_vbias:
                        mm(vps[0:nkz, :], ones1_sb[0:1, t0:t0 + nkz], wvb_sb,
                           start=False, stop=True)
                    vt = vp.tile([128, 256], dtb, tag="v", name="v")
                    nc.vector.tensor_copy(vt[0:nkz, :], vps[0:nkz, :])
                    vts[(i, jj)] = vt

            exps = {}  # (slot, jj) -> exp tile [nkz, 8, w]

            def S_slot(i):
                # scores via K=128 matmuls: stationary k4[g] (all 4 heads),
                # moving qz[g] slot p (head 4g+p's rows, zeros elsewhere).
                w, off = ws[i], offs[i]
                for jj in range(kts[i]):
                    nkz = min(128, w - 128 * jj)
                    t0 = off + 128 * jj
                    ti = kb[i] + jj
                    exp_t = ep.tile([128, 8, w], dtb, tag="exp", name="exp",
                                    padded_shape=[128, 8, 256])
                    if w > 128:
                        for g in range(2):
                            sp = psc.tile([128, 4, w], dtf, tag="sc", name="sc",
                                          padded_shape=[128, 4, 256])
                            for pp in range(0, 4, 2):
                                mm(sp[0:nkz, pp:pp + 2, 0:w],
                                   k4_sb[g][:, t0:t0 + nkz],
                                   qz_sb[g][:, pp:pp + 2, off:off + w],
                                   start=True, stop=True)
                            nc.scalar.activation(
                                exp_t[0:nkz, 4 * g:4 * g + 4, 0:w],
                                sp[0:nkz, :, 0:w], AF.Exp,
                                bias=bb_sb[0:nkz, 8 + ti:9 + ti], scale=SCALE)
                    else:
                        sp = psc.tile([128, 8, w], dtf, tag="sc", name="sc",
                                      padded_shape=[128, 8, 128])
                        nh = 2 if w > 64 else 4
                        for g in range(2):
                            for pp in range(0, 4, nh):
                                mm(sp[0:nkz, 4 * g + pp:4 * g + pp + nh, 0:w],
                                   k4_sb[g][:, t0:t0 + nkz],
                                   qz_sb[g][:, pp:pp + nh, off:off + w],
                                   start=True, stop=True)
                        nc.scalar.activation(
                            exp_t[0:nkz, :, 0:w], sp[0:nkz, :, 0:w], AF.Exp,
                            bias=bb_sb[0:nkz, 8 + ti:9 + ti], scale=SCALE)
                    exps[(i, jj)] = exp_t

            def D_slot(i):
                # den/ctx: per-band accumulation chains across k-tiles.  Each
                # 32-partition band holds an independent chain (per-element
                # has_written bits make band-disjoint chains in one bank safe);
                # the 4 bands' matmuls run concurrently via col tiling.
                w, kt, off = ws[i], kts[i], offs[i]
                dps = pd.tile([128, 2 * w], dtf, tag="dps", name="dps",
                              padded_shape=[128, 512])
                cps = pc.tile([128, 2 * w], dtf, tag="cps", name="cps",
                              padded_shape=[128, 512])
                for jj in range(kt):
                    nkz = min(128, w - 128 * jj)
                    exp_t = exps.pop((i, jj))
                    vt = vts.pop((i, jj))
                    first, last = jj == 0, jj == kt - 1
                    for j in range(4):
                        rh = exp_t[0:nkz, j:j + 5:4, 0:w]
                        mm(dps[32 * j:32 * j + 32, 0:2 * w],
                           ones_sb[0:nkz, 0:32], rh, start=first, stop=last,
                           tile_position=(0, 32 * j))
                    for g in range(2):
                        for j in range(4):
                            h = 4 * g + j
                            mm(cps[32 * j:32 * j + 32, g * w:(g + 1) * w],
                               vt[0:nkz, 32 * h:32 * h + 32],
                               exp_t[0:nkz, h, 0:w],
                               start=first and g == 0,
                               stop=last and g == 1,
                               tile_position=(0, 32 * j))
                rc = rp.tile([128, 2 * w], dtf, tag="rc", name="rc",
                             padded_shape=[128, 512])
                nc.vector.reciprocal_approx_fast(rc[:, 0:2 * w],
                                                 dps[:, 0:2 * w])
                for g in range(2):
                    nc.vector.tensor_mul(ctxT_sb[g][:, off:off + w],
                                         cps[:, g * w:(g + 1) * w],
                                         rc[:, g * w:(g + 1) * w])

            def C_group(gi):
                a, b = cgroups[gi]
                goff, gw = offs[a], ws[a] + ws[b]
                gs = slice(goff, goff + gw)
                for ft in range(2):
                    aps = pmm.tile([128, 512], dtf, tag="mm", name="mm")
                    mm(aps[:, :gw], wb_sb[:, WO0 + 128 * ft:WO0 + 128 * ft + 128],
                       ctxT_sb[0][:, gs], start=True, stop=False)
                    mm(aps[:, :gw], wb_sb[:, WO0 + 256 + 128 * ft:WO0 + 384 + 128 * ft],
                       ctxT_sb[1][:, gs], start=False, stop=True)
                    nc.vector.scalar_tensor_tensor(
                        rsap_sb[ft][:, gs], aps[:, :gw], bb_sb[:, 4 + ft:5 + ft],
                        hT_sb[ft][:, gs], OP.add, OP.add)
                for ft in range(2):
                    rps = pmm.tile([128, 512], dtf, tag="mm", name="mm")
                    mm(rps[:, :gw], wb_sb[:, WU0 + 128 * ft:WU0 + 128 * ft + 128],
                       rsap_sb[0][:, gs], start=True, stop=False)
                    mm(rps[:, :gw], wb_sb[:, WU0 + 256 + 128 * ft:WU0 + 384 + 128 * ft],
                       rsap_sb[1][:, gs], start=False, stop=True)
                    nc.scalar.activation(rsa_sb[ft][:, gs], rps[:, :gw],
                                         AF.Relu, bias=bb_sb[:, 6 + ft:7 + ft])
                vps = pmm.tile([1, 512], dtf, tag="mm", name="mm")
                mm(vps[0:1, :gw], wb_sb[:, WF0:WF0 + 1], rsa_sb[0][:, gs],
                   start=True, stop=False)
                mm(vps[0:1, :gw], wb_sb[:, WF0 + 1:WF0 + 2], rsa_sb[1][:, gs],
                   start=False, stop=True)
                vtmp = rp.tile([1, 512], dtf, tag="vt", name="vt")
                nc.vector.tensor_scalar_add(vtmp[0:1, 0:gw], vps[0:1, 0:gw],
                                            bb_sb[0:1, BB - 1:BB])
                nc.vector.scalar_tensor_tensor(
                    val_sb[0:1, gs], vtmp[0:1, 0:gw], 0.01, vtmp[0:1, 0:gw],
                    OP.mult, OP.max)
                for s in (a, b):
                    nc.vector.scalar_tensor_tensor(
                        vscr_sb[0:1, 0:ws[s]], val_sb[0:1, offs[s]:offs[s] + ws[s]],
                        1.0, m01_sb[0:1, offs[s]:offs[s] + ws[s]],
                        OP.mult, OP.mult, accum_out=out_sb[0:1, s:s + 1])

            def QZ_seg(c0, c1, engs):
                # band-slot copies q4 -> qz (same partitions, col shift);
                # g=0 and g=1 go to different trigger queues in parallel
                for g in range(2):
                    for p in range(4):
                        engs[g].dma_start(
                            out=qz_sb[g][32 * p:32 * p + 32, p, c0:c1],
                            in_=q4_sb[g][32 * p:32 * p + 32, c0:c1])

            # ---------------- emission schedule (priority order; the tile
            # scheduler dispatches by readiness, preferring earlier emission)
            spl1 = min(T, offs[2] if offs[2] > 0 else T)
            seg_bounds = sorted(set(min(x, T) for x in (offs[2], offs[4], T)))
            for g in range(2):
                for p in range(4):
                    eng = nc.gpsimd if (p % 2 == 0) else nc.vector
                    eng.memset(qz_sb[g][:, p, 0:spl1], 0.0)
            for g in range(2):
                for p in range(4):
                    if spl1 < T:
                        eng = nc.gpsimd if (p % 2 == 0) else nc.vector
                        eng.memset(qz_sb[g][:, p, spl1:T], 0.0)
            chunk_starts = list(range(0, T, 512))
            seg_prev = 0
            segq = []
            for se in seg_bounds:
                if se > seg_prev:
                    segq.append((seg_prev, se))
                    seg_prev = se
            seg_done = [False] * len(segq)
            seg_engs = [(nc.sync, nc.scalar), (nc.sync, nc.scalar),
                        (nc.gpsimd, nc.gpsimd)]

            def emit_segs(cols_done):
                for k, (a, b) in enumerate(segq):
                    if not seg_done[k] and b <= cols_done:
                        QZ_seg(a, b, seg_engs[min(k, 2)])
                        seg_done[k] = True

            A_chunk(chunk_starts[0])
            emit_segs(min(T, 512))
            V_slot(0); V_slot(1)
            S_slot(0)
            if len(chunk_starts) > 1:
                A_chunk(chunk_starts[1])
                emit_segs(min(T, 1024))
            D_slot(0)
            S_slot(1)
            V_slot(2); V_slot(3)
            S_slot(2)
            for c0 in chunk_starts[2:]:
                A_chunk(c0)
            emit_segs(T)
            D_slot(1)
            S_slot(3)
            V_slot(4); V_slot(5); V_slot(6); V_slot(7)
            D_slot(2)
            S_slot(4)
            D_slot(3)
            C_group(0)
            S_slot(5)
            D_slot(4)
            S_slot(6)
            D_slot(5)
            C_group(1)
            S_slot(7)
            D_slot(6)
            C_group(2)
            D_slot(7)
            C_group(3)
            nc.sync.dma_start(out=out_d, in_=out_sb)

    nc.compile()
    return nc


def get_program(plan, has_vbias):
    key = (plan["T"], plan["ws"], bool(has_vbias))
    if key not in _PROG_CACHE:
        _PROG_CACHE[key] = _build_program(key)
    return _PROG_CACHE[key]


# ---------------------------------------------------------------- host data
def _shared_inputs(W_in, b_in, W_qkv, b_qkv, W_o, b_o, W_out, b_out, W_v, b_v,
                   NKT):
    f32 = np.float32
    W_in = np.asarray(W_in, f32)
    b_qkv = np.asarray(b_qkv, f32)
    b_o, b_out = np.asarray(b_o, f32), np.asarray(b_out, f32)
    wb = np.zeros((128, WBC), f32)
    w_in_t = W_in[:, :256].T              # [256 in-feat, 256 out-feat]
    wb[:, WIN0:WIN0 + 256] = w_in_t[0:128]
    wb[:, WIN0 + 256:WIN0 + 512] = w_in_t[128:256]
    w_qk_t = np.asarray(W_qkv, f32)[:2 * E].T   # [256, 512]
    wb[:, WQK0:WQK0 + 512] = w_qk_t[0:128]
    wb[:, WQK0 + 512:WQK0 + 1024] = w_qk_t[128:256]
    w_v_t = np.asarray(W_qkv, f32)[2 * E:3 * E].T
    wb[:, WV0:WV0 + 256] = w_v_t[0:128]
    wb[:, WV0 + 256:WV0 + 512] = w_v_t[128:256]
    w_o_t = np.asarray(W_o, f32).T
    wb[:, WO0:WO0 + 256] = w_o_t[0:128]
    wb[:, WO0 + 256:WO0 + 512] = w_o_t[128:256]
    w_out_t = np.asarray(W_out, f32).T
    wb[:, WU0:WU0 + 256] = w_out_t[0:128]
    wb[:, WU0 + 256:WU0 + 512] = w_out_t[128:256]
    w_f_t = np.asarray(W_v, f32).T        # [256, 1]
    wb[:, WF0:WF0 + 1] = w_f_t[0:128]
    wb[:, WF0 + 1:WF0 + 2] = w_f_t[128:256]
    wr = np.stack([W_in[:, 256], np.asarray(b_in, f32)], axis=0)  # [2, 256]
    BB = 8 + NKT + 1
    bb = np.zeros((128, BB), np.float32)
    bb[:, 0:4] = b_qkv[:2 * E].reshape(4, 128).T
    bb[:, 4] = b_o[:128]; bb[:, 5] = b_o[128:]
    bb[:, 6] = b_out[:128]; bb[:, 7] = b_out[128:]
    bb[:, BB - 1] = float(np.asarray(b_v, f32).reshape(-1)[0])
    shared = {"wb": wb.astype(BF16), "wr": wr.astype(BF16), "bb": bb}
    has_vbias = bool(np.any(b_qkv[2 * E:] != 0))
    if has_vbias:
        shared["w_vb"] = b_qkv[2 * E:].reshape(1, 256).astype(BF16)
    return shared, has_vbias


def _core_inputs(plan, c, encoded_obs, shared):
    f32 = np.float32
    T, ws, offs, kts, kb, NKT = (plan["T"], plan["ws"], plan["offs"],
                                 plan["kts"], plan["kb"], plan["NKT"])
    a = plan["a"]
    xT = np.zeros((258, T), f32)
    m01 = np.zeros((1, T), f32)
    bb = shared["bb"].copy()
    p = np.arange(128)
    for i, s in enumerate(plan["slots"][c]):
        ai, w, off = int(a[s]), ws[i], offs[i]
        xT[0:256, off:off + ai] = np.asarray(encoded_obs[s, :ai, :], f32).T
        xT[256, off:off + ai] = ai / N
        xT[257, off:off + w] = 1.0
        m01[0, off:off + ai] = 1.0
        for jj in range(kts[i]):
            tok = 128 * jj + p
            bb[tok >= ai, 8 + kb[i] + jj] = NEG
    im = {"xT": xT.astype(BF16), "bb": bb, "mask01": m01}
    im["wb"] = shared["wb"]
    im["wr"] = shared["wr"]
    if "w_vb" in shared:
        im["w_vb"] = shared["w_vb"]
    return im


# ---------------------------------------------------------------- entry
def kernel(**inputs):
    global LAST_RESULT
    encoded_obs = np.asarray(inputs["encoded_obs"])
    actives = np.asarray(inputs["actives"]).reshape(-1)
    plan = _plan(actives)
    shared, has_vbias = _shared_inputs(
        inputs["W_in"], inputs["b_in"], inputs["W_qkv"], inputs["b_qkv"],
        inputs["W_o"], inputs["b_o"], inputs["W_out"], inputs["b_out"],
        inputs["W_v"], inputs["b_v"], plan["NKT"])
    nc = get_program(plan, has_vbias)
    in_maps = [_core_inputs(plan, c, encoded_obs, shared)
               for c in range(NCORES)]
    trace = bool(int(os.environ.get("KERNEL_TRACE", "0")))
    res = run_bass_kernel_spmd(nc, in_maps, core_ids=list(range(NCORES)),
                               trace=trace)
    LAST_RESULT = res
    out = np.zeros((B, 1), np.float32)
    for c in range(NCORES):
        vals = res.results[c]["val_out"].reshape(-1)
        for i, s in enumerate(plan["slots"][c]):
            out[s, 0] = vals[i]
    return out


# revision 22
# speedup vs baseline: 1.0462x; 1.0462x over previous
# Trainium2 Bass kernel for nn_Critic (RSA block critic over ragged agent sets).
#
# Strategy v2:
#  - Data-parallel over batch: 64 samples -> 8 cores x 8 samples, globally
#    sorted by length and snake-striped so all cores share ONE program.
#  - Activations feature-major ([feature, token]) bf16; fp32 PSUM accum.
#  - q/k keep their natural "4 heads per 32-partition band" layout straight
#    out of the QK projection; score matmuls are row-tiled (tile_position=
#    (32p, 0)) so 4 heads run concurrently in the PE array.
#  - All phase-B matmuls are single-shot (start&stop) PSUM groups; k-tile
#    accumulation for the den/ctx of two-tile slots happens on the DVE.
#    This avoids serialized per-bank accumulation chains.
#  - Few, large DMAs (one bf16 weight blob, column-split xT) on HWDGE
#    queues only (sync/scalar/vector); gpsimd does no DMA.
#  - ScalarE runs exp (big 1024-col instructions) + relu only; exp table
#    prefetched by a dummy exp at kernel start.
import math
import os

import numpy as np
import ml_dtypes

import concourse.bass as bass
import concourse.mybir as mybir
import concourse.tile as tile
from concourse import bacc
from concourse.bass_utils import run_bass_kernel_spmd

B, N, D, E, H, DH = 64, 256, 256, 256, 8, 32
NCORES, SPC = 8, 8
NEG = -1e9
PADW = 32
SCALE = 1.0 / math.sqrt(DH)
BF16 = ml_dtypes.bfloat16
AF = mybir.ActivationFunctionType
OP = mybir.AluOpType

# weight blob column offsets (bf16, [128, WBC])
WIN0, WQK0, WV0, WO0, WU0, WF0 = 0, 512, 1536, 2048, 2560, 3072
WBC = 3074

LAST_RESULT = None  # BassKernelResults of the most recent run (for test harness)


# ---------------------------------------------------------------- planning
def _plan(actives):
    a = np.asarray(actives).reshape(-1).astype(np.int64)
    assert a.shape == (B,)
    order = np.argsort(-a, kind="stable")
    slots = [[] for _ in range(NCORES)]
    for r, s in enumerate(order):
        stripe, pos = divmod(r, NCORES)
        c = pos if stripe % 2 == 0 else NCORES - 1 - pos
        slots[c].append(int(s))
    for c in range(NCORES):
        slots[c].sort(key=lambda s: -int(a[s]))
    ws = []
    for i in range(SPC):
        wi = max(int(a[slots[c][i]]) for c in range(NCORES))
        wi = max(PADW, ((wi + PADW - 1) // PADW) * PADW)
        ws.append(wi)
    kts = [(w + 127) // 128 for w in ws]
    offs = np.concatenate([[0], np.cumsum(ws)]).astype(int)
    kb = np.concatenate([[0], np.cumsum(kts)]).astype(int)
    return dict(
        a=a, slots=slots, ws=tuple(ws), kts=tuple(kts),
        offs=tuple(int(x) for x in offs[:-1]), T=int(offs[-1]),
        kb=tuple(int(x) for x in kb[:-1]), NKT=int(kb[-1]),
    )


# ---------------------------------------------------------------- program
_PROG_CACHE = {}


def _build_program(key):
    (T, ws, has_vbias) = key
    kts = tuple((w + 127) // 128 for w in ws)
    offs, kb = [], []
    o = k = 0
    for w, kt in zip(ws, kts):
        offs.append(o); kb.append(k); o += w; k += kt
    NKT = k
    BB = 8 + NKT + 1  # b_qk[0:4] b_oo[4:8] maskb[8:8+NKT] b_v[BB-1]
    dtb, dtf = mybir.dt.bfloat16, mybir.dt.float32
    cgroups = [(0, 1), (2, 3), (4, 5), (6, 7)]

    nc = bacc.Bacc("TRN2", target_bir_lowering=False, debug=False,
                   enable_asserts=False, num_devices=NCORES)

    def din(name, shape, dt):
        return nc.dram_tensor(name, shape, dt, kind="ExternalInput").ap()

    xT_d = din("xT", [258, T], dtb)
    wb_d = din("wb", [128, WBC], dtb)
    wr_d = din("wr", [2, 256], dtb)
    bb_d = din("bb", [128, BB], dtf)
    m01_d = din("mask01", [1, T], dtf)
    wvb_d = din("w_vb", [1, 256], dtb) if has_vbias else None
    out_d = nc.dram_tensor("val_out", [1, SPC], dtf, kind="ExternalOutput").ap()

    spl = min(T, 512)  # first xT column split (covers phase-A chunk 0)

    with tile.TileContext(nc) as tc:
        with (
            tc.tile_pool(name="const", bufs=1) as cp,
            tc.tile_pool(name="big", bufs=1) as bp,
            tc.tile_pool(name="vp", bufs=NKT) as vp,
            tc.tile_pool(name="ep", bufs=4) as ep,
            tc.tile_pool(name="rp", bufs=4) as rp,
            tc.tile_pool(name="pmm", bufs=2, space="PSUM") as pmm,
            tc.tile_pool(name="psc", bufs=2, space="PSUM") as psc,
            tc.tile_pool(name="pd", bufs=1, space="PSUM") as pd,
            tc.tile_pool(name="pc", bufs=1, space="PSUM") as pc,
        ):
            # ---- SBUF tiles
            wb_sb = cp.tile([128, WBC], dtb, tag="wb", name="wb")
            wr_sb = cp.tile([2, 256], dtb, tag="wr", name="wr")
            bb_sb = cp.tile([128, BB], dtf, tag="bb", name="bb")
            m01_sb = cp.tile([1, T], dtf, tag="m01", name="m01")
            ones_sb = cp.tile([128, 512], dtb, tag="ones", name="ones")
            dscr = cp.tile([1, 2], dtf, tag="dscr", name="dscr")
            xT_sb = [bp.tile([128, T], dtb, tag="xT0", name="xT0"),
                     bp.tile([128, T], dtb, tag="xT1", name="xT1"),
                     bp.tile([2, T], dtb, tag="xT2", name="xT2")]
            hT_sb = [bp.tile([128, T], dtb, tag=f"hT{f}", name=f"hT{f}")
                     for f in range(2)]
            q4_sb = [bp.tile([128, T], dtb, tag=f"q4{g}", name=f"q4{g}")
                     for g in range(2)]
            k4_sb = [bp.tile([128, T], dtb, tag=f"k4{g}", name=f"k4{g}")
                     for g in range(2)]
            # zero-padded per-head q: slot p holds head 4g+p's rows in band
            # 32p, zeros elsewhere -> K=128 score matmuls pick out one head
            # while sharing the k4 stationary (no base-0 relayout needed).
            qz_sb = [bp.tile([128, 4, T], dtb, tag=f"qz{g}", name=f"qz{g}")
                     for g in range(2)]
            ctxT_sb = [bp.tile([128, T], dtb, tag=f"cx{f}", name=f"cx{f}")
                       for f in range(2)]
            rsap_sb = [bp.tile([128, T], dtb, tag=f"rp{f}", name=f"rp{f}")
                       for f in range(2)]
            rsa_sb = [bp.tile([128, T], dtb, tag=f"rs{f}", name=f"rs{f}")
                      for f in range(2)]
            val_sb = bp.tile([1, T], dtf, tag="val", name="val")
            vscr_sb = bp.tile([1, 512], dtf, tag="vscr", name="vscr")
            out_sb = bp.tile([1, SPC], dtf, tag="out", name="out")
            if has_vbias:
                ones1_sb = cp.tile([1, T], dtb, tag="ones1", name="ones1")
                wvb_sb = cp.tile([1, 256], dtb, tag="wvb", name="wvb")

            # ---- DMA triggers (HWDGE only; order = need order)
            nc.scalar.dma_start(out=wb_sb[:, 0:WQK0], in_=wb_d[:, 0:WQK0])
            nc.scalar.dma_start(out=wb_sb[:, WQK0:WV0], in_=wb_d[:, WQK0:WV0])
            nc.gpsimd.dma_start(out=wr_sb, in_=wr_d)
            nc.gpsimd.dma_start(out=xT_sb[2], in_=xT_d[256:258, :])
            nc.sync.dma_start(out=xT_sb[0][:, 0:spl], in_=xT_d[0:128, 0:spl])
            nc.sync.dma_start(out=xT_sb[1][:, 0:spl], in_=xT_d[128:256, 0:spl])
            nc.sync.dma_start(out=bb_sb, in_=bb_d)
            if spl < T:
                nc.sync.dma_start(out=xT_sb[0][:, spl:T], in_=xT_d[0:128, spl:T])
                nc.sync.dma_start(out=xT_sb[1][:, spl:T], in_=xT_d[128:256, spl:T])
            nc.sync.dma_start(out=wb_sb[:, WV0:WBC], in_=wb_d[:, WV0:WBC])
            nc.sync.dma_start(out=m01_sb, in_=m01_d)
            if has_vbias:
                nc.sync.dma_start(out=wvb_sb, in_=wvb_d)
                nc.vector.memset(ones1_sb, 1.0)

            # exp table prefetch + constants
            nc.vector.memset(ones_sb, 1.0)
            nc.vector.memset(dscr[0:1, 0:1], 0.0)
            nc.scalar.activation(dscr[0:1, 1:2], dscr[0:1, 0:1], AF.Exp)

            mm = nc.tensor.matmul

            # PE warm-up: dense dummy matmuls on the ones tile while the
            # input DMAs stream, so HAM un-throttles before real work starts.
            wps = pmm.tile([128, 512], dtf, tag="mm", name="mm")
            for wi in range(6):
                mm(wps[0:64, 0:512], ones_sb[:, 0:64], ones_sb[:, 0:512],
                   start=True, stop=True)

            # ---------------- emission helpers
            def A_chunk(c0):
                cw = min(512, T - c0)
                cs = slice(c0, c0 + cw)
                for ft in range(2):
                    fs = slice(128 * ft, 128 * ft + 128)
                    hps = pmm.tile([128, 512], dtf, tag="mm", name="mm")
                    mm(hps[:, :cw], wb_sb[:, WIN0 + 128 * ft:WIN0 + 128 * ft + 128],
                       xT_sb[0][:, cs], start=True, stop=False)
                    mm(hps[:, :cw], wb_sb[:, WIN0 + 256 + 128 * ft:WIN0 + 384 + 128 * ft],
                       xT_sb[1][:, cs], start=False, stop=False)
                    mm(hps[:, :cw], wr_sb[:, fs], xT_sb[2][:, cs],
                       start=False, stop=True)
                    nc.scalar.activation(hT_sb[ft][:, cs], hps[:, :cw], AF.Relu)
                for m in range(4):
                    qps = pmm.tile([128, 512], dtf, tag="mm", name="mm")
                    mm(qps[:, :cw], wb_sb[:, WQK0 + 128 * m:WQK0 + 128 * m + 128],
                       hT_sb[0][:, cs], start=True, stop=False)
                    mm(qps[:, :cw], wb_sb[:, WQK0 + 512 + 128 * m:WQK0 + 640 + 128 * m],
                       hT_sb[1][:, cs], start=False, stop=True)
                    dst = (q4_sb if m < 2 else k4_sb)[m % 2]
                    nc.vector.tensor_scalar_add(dst[:, cs], qps[:, :cw],
                                                bb_sb[:, m:m + 1])

            vts = {}  # (slot, jj) -> bf16 v tile [nkz, 256]

            def V_slot(i):
                w, off = ws[i], offs[i]
                for jj in range(kts[i]):
                    nkz = min(128, w - 128 * jj)
                    t0 = off + 128 * jj
                    vps = pmm.tile([128, 256], dtf, tag="mm", name="mm")
                    mm(vps[0:nkz, :], hT_sb[0][:, t0:t0 + nkz],
                       wb_sb[:, WV0:WV0 + 256], start=True, stop=False)
                    mm(vps[0:nkz, :], hT_sb[1][:, t0:t0 + nkz],
                       wb_sb[:, WV0 + 256:WV0 + 512], start=False,
                       stop=not has_vbias)
                    if has_vbias:
                        mm(vps[0:nkz, :], ones1_sb[0:1, t0:t0 + nkz], wvb_sb,
                           start=False, stop=True)
                    vt = vp.tile([128, 256], dtb, tag="v", name="v")
                    nc.vector.tensor_copy(vt[0:nkz, :], vps[0:nkz, :])
                    vts[(i, jj)] = vt

            exps = {}  # (slot, jj) -> exp tile [nkz, 8, w]

            def S_slot(i):
                # scores via K=128 matmuls: stationary k4[g] (all 4 heads),
                # moving qz[g] slot p (head 4g+p's rows, zeros elsewhere).
                w, off = ws[i], offs[i]
                for jj in range(kts[i]):
                    nkz = min(128, w - 128 * jj)
                    t0 = off + 128 * jj
                    ti = kb[i] + jj
                    exp_t = ep.tile([128, 8, w], dtb, tag="exp", name="exp",
                                    padded_shape=[128, 8, 256])
                    if w > 128:
                        for g in range(2):
                            sp = psc.tile([128, 4, w], dtf, tag="sc", name="sc",
                                          padded_shape=[128, 4, 256])
                            for pp in range(0, 4, 2):
                                mm(sp[0:nkz, pp:pp + 2, 0:w],
                                   k4_sb[g][:, t0:t0 + nkz],
                                   qz_sb[g][:, pp:pp + 2, off:off + w],
                                   start=True, stop=True)
                            nc.scalar.activation(
                                exp_t[0:nkz, 4 * g:4 * g + 4, 0:w],
                                sp[0:nkz, :, 0:w], AF.Exp,
                                bias=bb_sb[0:nkz, 8 + ti:9 + ti], scale=SCALE)
                    else:
                        sp = psc.tile([128, 8, w], dtf, tag="sc", name="sc",
                                      padded_shape=[128, 8, 128])
                        nh = 2 if w > 64 else 4
                        for g in range(2):
                            for pp in range(0, 4, nh):
                                mm(sp[0:nkz, 4 * g + pp:4 * g + pp + nh, 0:w],
                                   k4_sb[g][:, t0:t0 + nkz],
                                   qz_sb[g][:, pp:pp + nh, off:off + w],
                                   start=True, stop=True)
                        nc.scalar.activation(
                            exp_t[0:nkz, :, 0:w], sp[0:nkz, :, 0:w], AF.Exp,
                            bias=bb_sb[0:nkz, 8 + ti:9 + ti], scale=SCALE)
                    exps[(i, jj)] = exp_t

            def D_slot(i):
                # den/ctx: per-band accumulation chains across k-tiles.  Each
                # 32-partition band holds an independent chain (per-element
                # has_written bits make band-disjoint chains in one bank safe);
                # the 4 bands' matmuls run concurrently via col tiling.
                w, kt, off = ws[i], kts[i], offs[i]
                dps = pd.tile([128, 2 * w], dtf, tag="dps", name="dps",
                              padded_shape=[128, 512])
                cps = pc.tile([128, 2 * w], dtf, tag="cps", name="cps",
                              padded_shape=[128, 512])
                for jj in range(kt):
                    nkz = min(128, w - 128 * jj)
                    exp_t = exps.pop((i, jj))
                    vt = vts.pop((i, jj))
                    first, last = jj == 0, jj == kt - 1
                    for j in range(4):
                        rh = exp_t[0:nkz, j:j + 5:4, 0:w]
                        mm(dps[32 * j:32 * j + 32, 0:2 * w],
                           ones_sb[0:nkz, 0:32], rh, start=first, stop=last,
                           tile_position=(0, 32 * j))
                    for g in range(2):
                        for j in range(4):
                            h = 4 * g + j
                            mm(cps[32 * j:32 * j + 32, g * w:(g + 1) * w],
                               vt[0:nkz, 32 * h:32 * h + 32],
                               exp_t[0:nkz, h, 0:w],
                               start=first and g == 0,
                               stop=last and g == 1,
                               tile_position=(0, 32 * j))
                rc = rp.tile([128, 2 * w], dtf, tag="rc", name="rc",
                             padded_shape=[128, 512])
                nc.vector.reciprocal_approx_fast(rc[:, 0:2 * w],
                                                 dps[:, 0:2 * w])
                for g in range(2):
                    nc.vector.tensor_mul(ctxT_sb[g][:, off:off + w],
                                         cps[:, g * w:(g + 1) * w],
                                         rc[:, g * w:(g + 1) * w])

            def C_group(gi):
                a, b = cgroups[gi]
                goff, gw = offs[a], ws[a] + ws[b]
                gs = slice(goff, goff + gw)
                for ft in range(2):
                    aps = pmm.tile([128, 512], dtf, tag="mm", name="mm")
                    mm(aps[:, :gw], wb_sb[:, WO0 + 128 * ft:WO0 + 128 * ft + 128],
                       ctxT_sb[0][:, gs], start=True, stop=False)
                    mm(aps[:, :gw], wb_sb[:, WO0 + 256 + 128 * ft:WO0 + 384 + 128 * ft],
                       ctxT_sb[1][:, gs], start=False, stop=True)
                    nc.vector.scalar_tensor_tensor(
                        rsap_sb[ft][:, gs], aps[:, :gw], bb_sb[:, 4 + ft:5 + ft],
                        hT_sb[ft][:, gs], OP.add, OP.add)
                for ft in range(2):
                    rps = pmm.tile([128, 512], dtf, tag="mm", name="mm")
                    mm(rps[:, :gw], wb_sb[:, WU0 + 128 * ft:WU0 + 128 * ft + 128],
                       rsap_sb[0][:, gs], start=True, stop=False)
                    mm(rps[:, :gw], wb_sb[:, WU0 + 256 + 128 * ft:WU0 + 384 + 128 * ft],
                       rsap_sb[1][:, gs], start=False, stop=True)
                    nc.scalar.activation(rsa_sb[ft][:, gs], rps[:, :gw],
                                         AF.Relu, bias=bb_sb[:, 6 + ft:7 + ft])
                vps = pmm.tile([1, 512], dtf, tag="mm", name="mm")
                mm(vps[0:1, :gw], wb_sb[:, WF0:WF0 + 1], rsa_sb[0][:, gs],
                   start=True, stop=False)
                mm(vps[0:1, :gw], wb_sb[:, WF0 + 1:WF0 + 2], rsa_sb[1][:, gs],
                   start=False, stop=True)
                vtmp = rp.tile([1, 512], dtf, tag="vt", name="vt")
                nc.vector.tensor_scalar_add(vtmp[0:1, 0:gw], vps[0:1, 0:gw],
                                            bb_sb[0:1, BB - 1:BB])
                nc.vector.scalar_tensor_tensor(
                    val_sb[0:1, gs], vtmp[0:1, 0:gw], 0.01, vtmp[0:1, 0:gw],
                    OP.mult, OP.max)
                for s in (a, b):
                    nc.vector.scalar_tensor_tensor(
                        vscr_sb[0:1, 0:ws[s]], val_sb[0:1, offs[s]:offs[s] + ws[s]],
                        1.0, m01_sb[0:1, offs[s]:offs[s] + ws[s]],
                        OP.mult, OP.mult, accum_out=out_sb[0:1, s:s + 1])

            def QZ_seg(c0, c1, engs):
                # band-slot copies q4 -> qz (same partitions, col shift);
                # g=0 and g=1 go to different trigger queues in parallel
                for g in range(2):
                    for p in range(4):
                        engs[g].dma_start(
                            out=qz_sb[g][32 * p:32 * p + 32, p, c0:c1],
                            in_=q4_sb[g][32 * p:32 * p + 32, c0:c1])

            # ---------------- emission schedule (priority order; the tile
            # scheduler dispatches by readiness, preferring earlier emission)
            spl1 = min(T, offs[2] if offs[2] > 0 else T)
            seg_bounds = sorted(set(min(x, T) for x in (offs[2], offs[4], T)))
            for g in range(2):
                for p in range(4):
                    eng = nc.gpsimd if (p % 2 == 0) else nc.vector
                    eng.memset(qz_sb[g][:, p, 0:spl1], 0.0)
            for g in range(2):
                for p in range(4):
                    if spl1 < T:
                        eng = nc.gpsimd if (p % 2 == 0) else nc.vector
                        eng.memset(qz_sb[g][:, p, spl1:T], 0.0)
            chunk_starts = list(range(0, T, 512))
            seg_prev = 0
            segq = []
            for se in seg_bounds:
                if se > seg_prev:
                    segq.append((seg_prev, se))
                    seg_prev = se
            seg_done = [False] * len(segq)
            seg_engs = [(nc.sync, nc.scalar), (nc.sync, nc.scalar),
                        (nc.gpsimd, nc.gpsimd)]

            def emit_segs(cols_done):
                for k, (a, b) in enumerate(segq):
                    if not seg_done[k] and b <= cols_done:
                        QZ_seg(a, b, seg_engs[min(k, 2)])
                        seg_done[k] = True

            A_chunk(chunk_starts[0])
            emit_segs(min(T, 512))
            V_slot(0); V_slot(1)
            S_slot(0)
            if len(chunk_starts) > 1:
                A_chunk(chunk_starts[1])
                emit_segs(min(T, 1024))
            D_slot(0)
            S_slot(1)
            for c0 in chunk_starts[2:]:
                A_chunk(c0)
            emit_segs(T)
            D_slot(1)
            V_slot(2); V_slot(3)
            S_slot(2)
            C_group(0)
            D_slot(2)
            S_slot(3); D_slot(3)
            V_slot(4); V_slot(5); V_slot(6); V_slot(7)
            C_group(1)
            S_slot(4); D_slot(4)
            S_slot(5); D_slot(5)
            C_group(2)
            S_slot(6); D_slot(6)
            S_slot(7); D_slot(7)
            C_group(3)
            nc.sync.dma_start(out=out_d, in_=out_sb)

    nc.compile()
    return nc


def get_program(plan, has_vbias):
    key = (plan["T"], plan["ws"], bool(has_vbias))
    if key not in _PROG_CACHE:
        _PROG_CACHE[key] = _build_program(key)
    return _PROG_CACHE[key]


# ---------------------------------------------------------------- host data
def _shared_inputs(W_in, b_in, W_qkv, b_qkv, W_o, b_o, W_out, b_out, W_v, b_v,
                   NKT):
    f32 = np.float32
    W_in = np.asarray(W_in, f32)
    b_qkv = np.asarray(b_qkv, f32)
    b_o, b_out = np.asarray(b_o, f32), np.asarray(b_out, f32)
    wb = np.zeros((128, WBC), f32)
    w_in_t = W_in[:, :256].T              # [256 in-feat, 256 out-feat]
    wb[:, WIN0:WIN0 + 256] = w_in_t[0:128]
    wb[:, WIN0 + 256:WIN0 + 512] = w_in_t[128:256]
    w_qk_t = np.asarray(W_qkv, f32)[:2 * E].T   # [256, 512]
    wb[:, WQK0:WQK0 + 512] = w_qk_t[0:128]
    wb[:, WQK0 + 512:WQK0 + 1024] = w_qk_t[128:256]
    w_v_t = np.asarray(W_qkv, f32)[2 * E:3 * E].T
    wb[:, WV0:WV0 + 256] = w_v_t[0:128]
    wb[:, WV0 + 256:WV0 + 512] = w_v_t[128:256]
    w_o_t = np.asarray(W_o, f32).T
    wb[:, WO0:WO0 + 256] = w_o_t[0:128]
    wb[:, WO0 + 256:WO0 + 512] = w_o_t[128:256]
    w_out_t = np.asarray(W_out, f32).T
    wb[:, WU0:WU0 + 256] = w_out_t[0:128]
    wb[:, WU0 + 256:WU0 + 512] = w_out_t[128:256]
    w_f_t = np.asarray(W_v, f32).T        # [256, 1]
    wb[:, WF0:WF0 + 1] = w_f_t[0:128]
    wb[:, WF0 + 1:WF0 + 2] = w_f_t[128:256]
    wr = np.stack([W_in[:, 256], np.asarray(b_in, f32)], axis=0)  # [2, 256]
    BB = 8 + NKT + 1
    bb = np.zeros((128, BB), np.float32)
    bb[:, 0:4] = b_qkv[:2 * E].reshape(4, 128).T
    bb[:, 4] = b_o[:128]; bb[:, 5] = b_o[128:]
    bb[:, 6] = b_out[:128]; bb[:, 7] = b_out[128:]
    bb[:, BB - 1] = float(np.asarray(b_v, f32).reshape(-1)[0])
    shared = {"wb": wb.astype(BF16), "wr": wr.astype(BF16), "bb": bb}
    has_vbias = bool(np.any(b_qkv[2 * E:] != 0))
    if has_vbias:
        shared["w_vb"] = b_qkv[2 * E:].reshape(1, 256).astype(BF16)
    return shared, has_vbias


def _core_inputs(plan, c, encoded_obs, shared):
    f32 = np.float32
    T, ws, offs, kts, kb, NKT = (plan["T"], plan["ws"], plan["offs"],
                                 plan["kts"], plan["kb"], plan["NKT"])
    a = plan["a"]
    xT = np.zeros((258, T), f32)
    m01 = np.zeros((1, T), f32)
    bb = shared["bb"].copy()
    p = np.arange(128)
    for i, s in enumerate(plan["slots"][c]):
        ai, w, off = int(a[s]), ws[i], offs[i]
        xT[0:256, off:off + ai] = np.asarray(encoded_obs[s, :ai, :], f32).T
        xT[256, off:off + ai] = ai / N
        xT[257, off:off + w] = 1.0
        m01[0, off:off + ai] = 1.0
        for jj in range(kts[i]):
            tok = 128 * jj + p
            bb[tok >= ai, 8 + kb[i] + jj] = NEG
    im = {"xT": xT.astype(BF16), "bb": bb, "mask01": m01}
    im["wb"] = shared["wb"]
    im["wr"] = shared["wr"]
    if "w_vb" in shared:
        im["w_vb"] = shared["w_vb"]
    return im


# ---------------------------------------------------------------- entry
def kernel(**inputs):
    global LAST_RESULT
    encoded_obs = np.asarray(inputs["encoded_obs"])
    actives = np.asarray(inputs["actives"]).reshape(-1)
    plan = _plan(actives)
    shared, has_vbias = _shared_inputs(
        inputs["W_in"], inputs["b_in"], inputs["W_qkv"], inputs["b_qkv"],
        inputs["W_o"], inputs["b_o"], inputs["W_out"], inputs["b_out"],
        inputs["W_v"], inputs["b_v"], plan["NKT"])
    nc = get_program(plan, has_vbias)
    in_maps = [_core_inputs(plan, c, encoded_obs, shared)
               for c in range(NCORES)]
    trace = bool(int(os.environ.get("KERNEL_TRACE", "0")))
    res = run_bass_kernel_spmd(nc, in_maps, core_ids=list(range(NCORES)),
                               trace=trace)
    LAST_RESULT = res
    out = np.zeros((B, 1), np.float32)
    for c in range(NCORES):
        vals = res.results[c]["val_out"].reshape(-1)
        for i, s in enumerate(plan["slots"][c]):
            out[s, 0] = vals[i]
    return out


# revision 23
# speedup vs baseline: 1.0789x; 1.0312x over previous
# Trainium2 Bass kernel for nn_Critic (RSA block critic over ragged agent sets).
#
# Strategy v2:
#  - Data-parallel over batch: 64 samples -> 8 cores x 8 samples, globally
#    sorted by length and snake-striped so all cores share ONE program.
#  - Activations feature-major ([feature, token]) bf16; fp32 PSUM accum.
#  - q/k keep their natural "4 heads per 32-partition band" layout straight
#    out of the QK projection; score matmuls are row-tiled (tile_position=
#    (32p, 0)) so 4 heads run concurrently in the PE array.
#  - All phase-B matmuls are single-shot (start&stop) PSUM groups; k-tile
#    accumulation for the den/ctx of two-tile slots happens on the DVE.
#    This avoids serialized per-bank accumulation chains.
#  - Few, large DMAs (one bf16 weight blob, column-split xT) on HWDGE
#    queues only (sync/scalar/vector); gpsimd does no DMA.
#  - ScalarE runs exp (big 1024-col instructions) + relu only; exp table
#    prefetched by a dummy exp at kernel start.
import math
import os

import numpy as np
import ml_dtypes

import concourse.bass as bass
import concourse.mybir as mybir
import concourse.tile as tile
from concourse import bacc
from concourse.bass_utils import run_bass_kernel_spmd

B, N, D, E, H, DH = 64, 256, 256, 256, 8, 32
NCORES, SPC = 8, 8
NEG = -1e9
PADW = 32
SCALE = 1.0 / math.sqrt(DH)
BF16 = ml_dtypes.bfloat16
AF = mybir.ActivationFunctionType
OP = mybir.AluOpType

# weight blob column offsets (bf16, [128, WBC])
WIN0, WQK0, WV0, WO0, WU0, WF0 = 0, 512, 1536, 2048, 2560, 3072
WBC = 3074

LAST_RESULT = None  # BassKernelResults of the most recent run (for test harness)


# ---------------------------------------------------------------- planning
def _plan(actives):
    a = np.asarray(actives).reshape(-1).astype(np.int64)
    assert a.shape == (B,)
    order = np.argsort(-a, kind="stable")
    slots = [[] for _ in range(NCORES)]
    for r, s in enumerate(order):
        stripe, pos = divmod(r, NCORES)
        c = pos if stripe % 2 == 0 else NCORES - 1 - pos
        slots[c].append(int(s))
    for c in range(NCORES):
        slots[c].sort(key=lambda s: -int(a[s]))
    ws = []
    for i in range(SPC):
        wi = max(int(a[slots[c][i]]) for c in range(NCORES))
        wi = max(PADW, ((wi + PADW - 1) // PADW) * PADW)
        ws.append(wi)
    kts = [(w + 127) // 128 for w in ws]
    offs = np.concatenate([[0], np.cumsum(ws)]).astype(int)
    kb = np.concatenate([[0], np.cumsum(kts)]).astype(int)
    return dict(
        a=a, slots=slots, ws=tuple(ws), kts=tuple(kts),
        offs=tuple(int(x) for x in offs[:-1]), T=int(offs[-1]),
        kb=tuple(int(x) for x in kb[:-1]), NKT=int(kb[-1]),
    )


# ---------------------------------------------------------------- program
_PROG_CACHE = {}


def _build_program(key):
    (T, ws, has_vbias) = key
    kts = tuple((w + 127) // 128 for w in ws)
    offs, kb = [], []
    o = k = 0
    for w, kt in zip(ws, kts):
        offs.append(o); kb.append(k); o += w; k += kt
    NKT = k
    BB = 8 + NKT + 1  # b_qk[0:4] b_oo[4:8] maskb[8:8+NKT] b_v[BB-1]
    dtb, dtf = mybir.dt.bfloat16, mybir.dt.float32
    cgroups = [(0, 1), (2, 3), (4, 5), (6, 7)]

    nc = bacc.Bacc("TRN2", target_bir_lowering=False, debug=False,
                   enable_asserts=False, num_devices=NCORES)

    def din(name, shape, dt):
        return nc.dram_tensor(name, shape, dt, kind="ExternalInput").ap()

    xT_d = din("xT", [258, T], dtb)
    wb_d = din("wb", [128, WBC], dtb)
    wr_d = din("wr", [2, 256], dtb)
    bb_d = din("bb", [128, BB], dtf)
    m01_d = din("mask01", [1, T], dtf)
    wvb_d = din("w_vb", [1, 256], dtb) if has_vbias else None
    out_d = nc.dram_tensor("val_out", [1, SPC], dtf, kind="ExternalOutput").ap()

    spl = min(T, 512)  # first xT column split (covers phase-A chunk 0)

    with tile.TileContext(nc) as tc:
        with (
            tc.tile_pool(name="const", bufs=1) as cp,
            tc.tile_pool(name="big", bufs=1) as bp,
            tc.tile_pool(name="vp", bufs=NKT) as vp,
            tc.tile_pool(name="ep", bufs=4) as ep,
            tc.tile_pool(name="rp", bufs=4) as rp,
            tc.tile_pool(name="pmm", bufs=2, space="PSUM") as pmm,
            tc.tile_pool(name="psc", bufs=2, space="PSUM") as psc,
            tc.tile_pool(name="pd", bufs=1, space="PSUM") as pd,
            tc.tile_pool(name="pc", bufs=1, space="PSUM") as pc,
        ):
            # ---- SBUF tiles
            wb_sb = cp.tile([128, WBC], dtb, tag="wb", name="wb")
            wr_sb = cp.tile([2, 256], dtb, tag="wr", name="wr")
            bb_sb = cp.tile([128, BB], dtf, tag="bb", name="bb")
            m01_sb = cp.tile([1, T], dtf, tag="m01", name="m01")
            ones_sb = cp.tile([128, 512], dtb, tag="ones", name="ones")
            dscr = cp.tile([1, 2], dtf, tag="dscr", name="dscr")
            xT_sb = [bp.tile([128, T], dtb, tag="xT0", name="xT0"),
                     bp.tile([128, T], dtb, tag="xT1", name="xT1"),
                     bp.tile([2, T], dtb, tag="xT2", name="xT2")]
            hT_sb = [bp.tile([128, T], dtb, tag=f"hT{f}", name=f"hT{f}")
                     for f in range(2)]
            q4_sb = [bp.tile([128, T], dtb, tag=f"q4{g}", name=f"q4{g}")
                     for g in range(2)]
            k4_sb = [bp.tile([128, T], dtb, tag=f"k4{g}", name=f"k4{g}")
                     for g in range(2)]
            # zero-padded per-head q: slot p holds head 4g+p's rows in band
            # 32p, zeros elsewhere -> K=128 score matmuls pick out one head
            # while sharing the k4 stationary (no base-0 relayout needed).
            qz_sb = [bp.tile([128, 4, T], dtb, tag=f"qz{g}", name=f"qz{g}")
                     for g in range(2)]
            ctxT_sb = [bp.tile([128, T], dtb, tag=f"cx{f}", name=f"cx{f}")
                       for f in range(2)]
            rsap_sb = [bp.tile([128, T], dtb, tag=f"rp{f}", name=f"rp{f}")
                       for f in range(2)]
            rsa_sb = [bp.tile([128, T], dtb, tag=f"rs{f}", name=f"rs{f}")
                      for f in range(2)]
            val_sb = bp.tile([1, T], dtf, tag="val", name="val")
            vscr_sb = bp.tile([1, 512], dtf, tag="vscr", name="vscr")
            out_sb = bp.tile([1, SPC], dtf, tag="out", name="out")
            if has_vbias:
                ones1_sb = cp.tile([1, T], dtb, tag="ones1", name="ones1")
                wvb_sb = cp.tile([1, 256], dtb, tag="wvb", name="wvb")

            # ---- DMA triggers (HWDGE only; order = need order)
            nc.scalar.dma_start(out=wb_sb[:, 0:WQK0], in_=wb_d[:, 0:WQK0])
            nc.scalar.dma_start(out=wb_sb[:, WQK0:WV0], in_=wb_d[:, WQK0:WV0])
            nc.gpsimd.dma_start(out=wr_sb, in_=wr_d)
            nc.gpsimd.dma_start(out=xT_sb[2], in_=xT_d[256:258, :])
            nc.sync.dma_start(out=xT_sb[0][:, 0:spl], in_=xT_d[0:128, 0:spl])
            nc.sync.dma_start(out=xT_sb[1][:, 0:spl], in_=xT_d[128:256, 0:spl])
            nc.sync.dma_start(out=bb_sb, in_=bb_d)
            if spl < T:
                nc.sync.dma_start(out=xT_sb[0][:, spl:T], in_=xT_d[0:128, spl:T])
                nc.sync.dma_start(out=xT_sb[1][:, spl:T], in_=xT_d[128:256, spl:T])
            nc.sync.dma_start(out=wb_sb[:, WV0:WBC], in_=wb_d[:, WV0:WBC])
            nc.sync.dma_start(out=m01_sb, in_=m01_d)
            if has_vbias:
                nc.sync.dma_start(out=wvb_sb, in_=wvb_d)
                nc.vector.memset(ones1_sb, 1.0)

            # exp table prefetch + constants
            nc.vector.memset(ones_sb, 1.0)
            nc.vector.memset(dscr[0:1, 0:1], 0.0)
            nc.scalar.activation(dscr[0:1, 1:2], dscr[0:1, 0:1], AF.Exp)

            mm = nc.tensor.matmul

            # PE warm-up: dense dummy matmuls on the ones tile while the
            # input DMAs stream, so HAM un-throttles before real work starts.
            wps = pmm.tile([128, 512], dtf, tag="mm", name="mm")
            for wi in range(6):
                mm(wps[0:64, 0:512], ones_sb[:, 0:64], ones_sb[:, 0:512],
                   start=True, stop=True)

            # ---------------- emission helpers
            def A_chunk(c0):
                cw = min(512, T - c0)
                cs = slice(c0, c0 + cw)
                for ft in range(2):
                    fs = slice(128 * ft, 128 * ft + 128)
                    hps = pmm.tile([128, 512], dtf, tag="mm", name="mm")
                    mm(hps[:, :cw], wb_sb[:, WIN0 + 128 * ft:WIN0 + 128 * ft + 128],
                       xT_sb[0][:, cs], start=True, stop=False)
                    mm(hps[:, :cw], wb_sb[:, WIN0 + 256 + 128 * ft:WIN0 + 384 + 128 * ft],
                       xT_sb[1][:, cs], start=False, stop=False)
                    mm(hps[:, :cw], wr_sb[:, fs], xT_sb[2][:, cs],
                       start=False, stop=True)
                    nc.scalar.activation(hT_sb[ft][:, cs], hps[:, :cw], AF.Relu)
                for m in range(4):
                    qps = pmm.tile([128, 512], dtf, tag="mm", name="mm")
                    mm(qps[:, :cw], wb_sb[:, WQK0 + 128 * m:WQK0 + 128 * m + 128],
                       hT_sb[0][:, cs], start=True, stop=False)
                    mm(qps[:, :cw], wb_sb[:, WQK0 + 512 + 128 * m:WQK0 + 640 + 128 * m],
                       hT_sb[1][:, cs], start=False, stop=True)
                    dst = (q4_sb if m < 2 else k4_sb)[m % 2]
                    nc.vector.tensor_scalar_add(dst[:, cs], qps[:, :cw],
                                                bb_sb[:, m:m + 1])

            vts = {}  # (slot, jj) -> bf16 v tile [nkz, 256]

            def V_slot(i):
                w, off = ws[i], offs[i]
                for jj in range(kts[i]):
                    nkz = min(128, w - 128 * jj)
                    t0 = off + 128 * jj
                    vps = pmm.tile([128, 256], dtf, tag="mm", name="mm")
                    mm(vps[0:nkz, :], hT_sb[0][:, t0:t0 + nkz],
                       wb_sb[:, WV0:WV0 + 256], start=True, stop=False)
                    mm(vps[0:nkz, :], hT_sb[1][:, t0:t0 + nkz],
                       wb_sb[:, WV0 + 256:WV0 + 512], start=False,
                       stop=not has_vbias)
                    if has_vbias:
                        mm(vps[0:nkz, :], ones1_sb[0:1, t0:t0 + nkz], wvb_sb,
                           start=False, stop=True)
                    vt = vp.tile([128, 256], dtb, tag="v", name="v")
                    nc.vector.tensor_copy(vt[0:nkz, :], vps[0:nkz, :])
                    vts[(i, jj)] = vt

            exps = {}  # (slot, jj) -> exp tile [nkz, 8, w]

            def S_slot(i):
                # scores via K=128 matmuls: stationary k4[g] (all 4 heads),
                # moving qz[g] slot p (head 4g+p's rows, zeros elsewhere).
                w, off = ws[i], offs[i]
                for jj in range(kts[i]):
                    nkz = min(128, w - 128 * jj)
                    t0 = off + 128 * jj
                    ti = kb[i] + jj
                    exp_t = ep.tile([128, 8, w], dtb, tag="exp", name="exp",
                                    padded_shape=[128, 8, 256])
                    if w > 128:
                        for g in range(2):
                            sp = psc.tile([128, 4, w], dtf, tag="sc", name="sc",
                                          padded_shape=[128, 4, 256])
                            for pp in range(0, 4, 2):
                                mm(sp[0:nkz, pp:pp + 2, 0:w],
                                   k4_sb[g][:, t0:t0 + nkz],
                                   qz_sb[g][:, pp:pp + 2, off:off + w],
                                   start=True, stop=True)
                            nc.scalar.activation(
                                exp_t[0:nkz, 4 * g:4 * g + 4, 0:w],
                                sp[0:nkz, :, 0:w], AF.Exp,
                                bias=bb_sb[0:nkz, 8 + ti:9 + ti], scale=SCALE)
                    else:
                        sp = psc.tile([128, 8, w], dtf, tag="sc", name="sc",
                                      padded_shape=[128, 8, 128])
                        nh = 2 if w > 64 else 4
                        for g in range(2):
                            for pp in range(0, 4, nh):
                                mm(sp[0:nkz, 4 * g + pp:4 * g + pp + nh, 0:w],
                                   k4_sb[g][:, t0:t0 + nkz],
                                   qz_sb[g][:, pp:pp + nh, off:off + w],
                                   start=True, stop=True)
                        nc.scalar.activation(
                            exp_t[0:nkz, :, 0:w], sp[0:nkz, :, 0:w], AF.Exp,
                            bias=bb_sb[0:nkz, 8 + ti:9 + ti], scale=SCALE)
                    exps[(i, jj)] = exp_t

            def D_slot(i):
                # den/ctx: per-band accumulation chains across k-tiles.  Each
                # 32-partition band holds an independent chain (per-element
                # has_written bits make band-disjoint chains in one bank safe);
                # the 4 bands' matmuls run concurrently via col tiling.
                w, kt, off = ws[i], kts[i], offs[i]
                dps = pd.tile([128, 2 * w], dtf, tag="dps", name="dps",
                              padded_shape=[128, 512])
                cps = pc.tile([128, 2 * w], dtf, tag="cps", name="cps",
                              padded_shape=[128, 512])
                for jj in range(kt):
                    nkz = min(128, w - 128 * jj)
                    exp_t = exps.pop((i, jj))
                    vt = vts.pop((i, jj))
                    first, last = jj == 0, jj == kt - 1
                    for j in range(4):
                        rh = exp_t[0:nkz, j:j + 5:4, 0:w]
                        mm(dps[32 * j:32 * j + 32, 0:2 * w],
                           ones_sb[0:nkz, 0:32], rh, start=first, stop=last,
                           tile_position=(0, 32 * j))
                    for g in range(2):
                        for j in range(4):
                            h = 4 * g + j
                            mm(cps[32 * j:32 * j + 32, g * w:(g + 1) * w],
                               vt[0:nkz, 32 * h:32 * h + 32],
                               exp_t[0:nkz, h, 0:w],
                               start=first and g == 0,
                               stop=last and g == 1,
                               tile_position=(0, 32 * j))
                rc = rp.tile([128, 2 * w], dtf, tag="rc", name="rc",
                             padded_shape=[128, 512])
                nc.vector.reciprocal_approx_fast(rc[:, 0:2 * w],
                                                 dps[:, 0:2 * w])
                for g in range(2):
                    nc.vector.tensor_mul(ctxT_sb[g][:, off:off + w],
                                         cps[:, g * w:(g + 1) * w],
                                         rc[:, g * w:(g + 1) * w])

            def C_group(gi):
                a, b = cgroups[gi]
                goff, gw = offs[a], ws[a] + ws[b]
                gs = slice(goff, goff + gw)
                for ft in range(2):
                    aps = pmm.tile([128, 512], dtf, tag="mm", name="mm")
                    mm(aps[:, :gw], wb_sb[:, WO0 + 128 * ft:WO0 + 128 * ft + 128],
                       ctxT_sb[0][:, gs], start=True, stop=False)
                    mm(aps[:, :gw], wb_sb[:, WO0 + 256 + 128 * ft:WO0 + 384 + 128 * ft],
                       ctxT_sb[1][:, gs], start=False, stop=True)
                    nc.vector.scalar_tensor_tensor(
                        rsap_sb[ft][:, gs], aps[:, :gw], bb_sb[:, 4 + ft:5 + ft],
                        hT_sb[ft][:, gs], OP.add, OP.add)
                for ft in range(2):
                    rps = pmm.tile([128, 512], dtf, tag="mm", name="mm")
                    mm(rps[:, :gw], wb_sb[:, WU0 + 128 * ft:WU0 + 128 * ft + 128],
                       rsap_sb[0][:, gs], start=True, stop=False)
                    mm(rps[:, :gw], wb_sb[:, WU0 + 256 + 128 * ft:WU0 + 384 + 128 * ft],
                       rsap_sb[1][:, gs], start=False, stop=True)
                    nc.scalar.activation(rsa_sb[ft][:, gs], rps[:, :gw],
                                         AF.Relu, bias=bb_sb[:, 6 + ft:7 + ft])
                vps = pmm.tile([1, 512], dtf, tag="mm", name="mm")
                mm(vps[0:1, :gw], wb_sb[:, WF0:WF0 + 1], rsa_sb[0][:, gs],
                   start=True, stop=False)
                mm(vps[0:1, :gw], wb_sb[:, WF0 + 1:WF0 + 2], rsa_sb[1][:, gs],
                   start=False, stop=True)
                vtmp = rp.tile([1, 512], dtf, tag="vt", name="vt")
                nc.vector.tensor_scalar_add(vtmp[0:1, 0:gw], vps[0:1, 0:gw],
                                            bb_sb[0:1, BB - 1:BB])
                nc.vector.scalar_tensor_tensor(
                    val_sb[0:1, gs], vtmp[0:1, 0:gw], 0.01, vtmp[0:1, 0:gw],
                    OP.mult, OP.max)
                for s in (a, b):
                    nc.vector.scalar_tensor_tensor(
                        vscr_sb[0:1, 0:ws[s]], val_sb[0:1, offs[s]:offs[s] + ws[s]],
                        1.0, m01_sb[0:1, offs[s]:offs[s] + ws[s]],
                        OP.mult, OP.mult, accum_out=out_sb[0:1, s:s + 1])

            def QZ_seg(c0, c1, engs):
                # band-slot copies q4 -> qz (same partitions, col shift);
                # g=0 and g=1 go to different trigger queues in parallel
                for g in range(2):
                    for p in range(4):
                        engs[g].dma_start(
                            out=qz_sb[g][32 * p:32 * p + 32, p, c0:c1],
                            in_=q4_sb[g][32 * p:32 * p + 32, c0:c1])

            # ---------------- emission schedule (priority order; the tile
            # scheduler dispatches by readiness, preferring earlier emission)
            spl1 = min(T, offs[2] if offs[2] > 0 else T)
            seg_bounds = sorted(set(min(x, T) for x in (offs[2], offs[4], T)))
            for g in range(2):
                for p in range(4):
                    eng = nc.gpsimd if (p % 2 == 0) else nc.vector
                    eng.memset(qz_sb[g][:, p, 0:spl1], 0.0)
            for g in range(2):
                for p in range(4):
                    if spl1 < T:
                        eng = nc.gpsimd if (p % 2 == 0) else nc.vector
                        eng.memset(qz_sb[g][:, p, spl1:T], 0.0)
            chunk_starts = list(range(0, T, 512))
            seg_prev = 0
            segq = []
            for se in seg_bounds:
                if se > seg_prev:
                    segq.append((seg_prev, se))
                    seg_prev = se
            seg_done = [False] * len(segq)
            seg_engs = [(nc.sync, nc.sync), (nc.sync, nc.sync),
                        (nc.sync, nc.sync)]

            def emit_segs(cols_done):
                for k, (a, b) in enumerate(segq):
                    if not seg_done[k] and b <= cols_done:
                        QZ_seg(a, b, seg_engs[min(k, 2)])
                        seg_done[k] = True

            A_chunk(chunk_starts[0])
            emit_segs(min(T, 512))
            V_slot(0); V_slot(1)
            S_slot(0)
            if len(chunk_starts) > 1:
                A_chunk(chunk_starts[1])
                emit_segs(min(T, 1024))
            D_slot(0)
            S_slot(1)
            for c0 in chunk_starts[2:]:
                A_chunk(c0)
            emit_segs(T)
            D_slot(1)
            V_slot(2); V_slot(3)
            S_slot(2)
            C_group(0)
            D_slot(2)
            S_slot(3); D_slot(3)
            V_slot(4); V_slot(5); V_slot(6); V_slot(7)
            C_group(1)
            S_slot(4); D_slot(4)
            S_slot(5); D_slot(5)
            C_group(2)
            S_slot(6); D_slot(6)
            S_slot(7); D_slot(7)
            C_group(3)
            nc.sync.dma_start(out=out_d, in_=out_sb)

    nc.compile()
    return nc


def get_program(plan, has_vbias):
    key = (plan["T"], plan["ws"], bool(has_vbias))
    if key not in _PROG_CACHE:
        _PROG_CACHE[key] = _build_program(key)
    return _PROG_CACHE[key]


# ---------------------------------------------------------------- host data
def _shared_inputs(W_in, b_in, W_qkv, b_qkv, W_o, b_o, W_out, b_out, W_v, b_v,
                   NKT):
    f32 = np.float32
    W_in = np.asarray(W_in, f32)
    b_qkv = np.asarray(b_qkv, f32)
    b_o, b_out = np.asarray(b_o, f32), np.asarray(b_out, f32)
    wb = np.zeros((128, WBC), f32)
    w_in_t = W_in[:, :256].T              # [256 in-feat, 256 out-feat]
    wb[:, WIN0:WIN0 + 256] = w_in_t[0:128]
    wb[:, WIN0 + 256:WIN0 + 512] = w_in_t[128:256]
    w_qk_t = np.asarray(W_qkv, f32)[:2 * E].T   # [256, 512]
    wb[:, WQK0:WQK0 + 512] = w_qk_t[0:128]
    wb[:, WQK0 + 512:WQK0 + 1024] = w_qk_t[128:256]
    w_v_t = np.asarray(W_qkv, f32)[2 * E:3 * E].T
    wb[:, WV0:WV0 + 256] = w_v_t[0:128]
    wb[:, WV0 + 256:WV0 + 512] = w_v_t[128:256]
    w_o_t = np.asarray(W_o, f32).T
    wb[:, WO0:WO0 + 256] = w_o_t[0:128]
    wb[:, WO0 + 256:WO0 + 512] = w_o_t[128:256]
    w_out_t = np.asarray(W_out, f32).T
    wb[:, WU0:WU0 + 256] = w_out_t[0:128]
    wb[:, WU0 + 256:WU0 + 512] = w_out_t[128:256]
    w_f_t = np.asarray(W_v, f32).T        # [256, 1]
    wb[:, WF0:WF0 + 1] = w_f_t[0:128]
    wb[:, WF0 + 1:WF0 + 2] = w_f_t[128:256]
    wr = np.stack([W_in[:, 256], np.asarray(b_in, f32)], axis=0)  # [2, 256]
    BB = 8 + NKT + 1
    bb = np.zeros((128, BB), np.float32)
    bb[:, 0:4] = b_qkv[:2 * E].reshape(4, 128).T
    bb[:, 4] = b_o[:128]; bb[:, 5] = b_o[128:]
    bb[:, 6] = b_out[:128]; bb[:, 7] = b_out[128:]
    bb[:, BB - 1] = float(np.asarray(b_v, f32).reshape(-1)[0])
    shared = {"wb": wb.astype(BF16), "wr": wr.astype(BF16), "bb": bb}
    has_vbias = bool(np.any(b_qkv[2 * E:] != 0))
    if has_vbias:
        shared["w_vb"] = b_qkv[2 * E:].reshape(1, 256).astype(BF16)
    return shared, has_vbias


def _core_inputs(plan, c, encoded_obs, shared):
    f32 = np.float32
    T, ws, offs, kts, kb, NKT = (plan["T"], plan["ws"], plan["offs"],
                                 plan["kts"], plan["kb"], plan["NKT"])
    a = plan["a"]
    xT = np.zeros((258, T), f32)
    m01 = np.zeros((1, T), f32)
    bb = shared["bb"].copy()
    p = np.arange(128)
    for i, s in enumerate(plan["slots"][c]):
        ai, w, off = int(a[s]), ws[i], offs[i]
        xT[0:256, off:off + ai] = np.asarray(encoded_obs[s, :ai, :], f32).T
        xT[256, off:off + ai] = ai / N
        xT[257, off:off + w] = 1.0
        m01[0, off:off + ai] = 1.0
        for jj in range(kts[i]):
            tok = 128 * jj + p
            bb[tok >= ai, 8 + kb[i] + jj] = NEG
    im = {"xT": xT.astype(BF16), "bb": bb, "mask01": m01}
    im["wb"] = shared["wb"]
    im["wr"] = shared["wr"]
    if "w_vb" in shared:
        im["w_vb"] = shared["w_vb"]
    return im


# ---------------------------------------------------------------- entry
def kernel(**inputs):
    global LAST_RESULT
    encoded_obs = np.asarray(inputs["encoded_obs"])
    actives = np.asarray(inputs["actives"]).reshape(-1)
    plan = _plan(actives)
    shared, has_vbias = _shared_inputs(
        inputs["W_in"], inputs["b_in"], inputs["W_qkv"], inputs["b_qkv"],
        inputs["W_o"], inputs["b_o"], inputs["W_out"], inputs["b_out"],
        inputs["W_v"], inputs["b_v"], plan["NKT"])
    nc = get_program(plan, has_vbias)
    in_maps = [_core_inputs(plan, c, encoded_obs, shared)
               for c in range(NCORES)]
    trace = bool(int(os.environ.get("KERNEL_TRACE", "0")))
    res = run_bass_kernel_spmd(nc, in_maps, core_ids=list(range(NCORES)),
                               trace=trace)
    LAST_RESULT = res
    out = np.zeros((B, 1), np.float32)
    for c in range(NCORES):
        vals = res.results[c]["val_out"].reshape(-1)
        for i, s in enumerate(plan["slots"][c]):
            out[s, 0] = vals[i]
    return out


# revision 24
# speedup vs baseline: 1.1321x; 1.0494x over previous
# Trainium2 Bass kernel for nn_Critic (RSA block critic over ragged agent sets).
#
# Strategy v2:
#  - Data-parallel over batch: 64 samples -> 8 cores x 8 samples, globally
#    sorted by length and snake-striped so all cores share ONE program.
#  - Activations feature-major ([feature, token]) bf16; fp32 PSUM accum.
#  - q/k keep their natural "4 heads per 32-partition band" layout straight
#    out of the QK projection; score matmuls are row-tiled (tile_position=
#    (32p, 0)) so 4 heads run concurrently in the PE array.
#  - All phase-B matmuls are single-shot (start&stop) PSUM groups; k-tile
#    accumulation for the den/ctx of two-tile slots happens on the DVE.
#    This avoids serialized per-bank accumulation chains.
#  - Few, large DMAs (one bf16 weight blob, column-split xT) on HWDGE
#    queues only (sync/scalar/vector); gpsimd does no DMA.
#  - ScalarE runs exp (big 1024-col instructions) + relu only; exp table
#    prefetched by a dummy exp at kernel start.
import math
import os

import numpy as np
import ml_dtypes

import concourse.bass as bass
import concourse.mybir as mybir
import concourse.tile as tile
from concourse import bacc
from concourse.bass_utils import run_bass_kernel_spmd

B, N, D, E, H, DH = 64, 256, 256, 256, 8, 32
NCORES, SPC = 8, 8
NEG = -1e9
PADW = 32
SCALE = 1.0 / math.sqrt(DH)
BF16 = ml_dtypes.bfloat16
AF = mybir.ActivationFunctionType
OP = mybir.AluOpType

# weight blob column offsets (bf16, [128, WBC])
WIN0, WQK0, WV0, WO0, WU0, WF0 = 0, 512, 1536, 2048, 2560, 3072
WBC = 3074

LAST_RESULT = None  # BassKernelResults of the most recent run (for test harness)


# ---------------------------------------------------------------- planning
def _plan(actives):
    a = np.asarray(actives).reshape(-1).astype(np.int64)
    assert a.shape == (B,)
    order = np.argsort(-a, kind="stable")
    slots = [[] for _ in range(NCORES)]
    for r, s in enumerate(order):
        stripe, pos = divmod(r, NCORES)
        c = pos if stripe % 2 == 0 else NCORES - 1 - pos
        slots[c].append(int(s))
    for c in range(NCORES):
        slots[c].sort(key=lambda s: -int(a[s]))
    ws = []
    for i in range(SPC):
        wi = max(int(a[slots[c][i]]) for c in range(NCORES))
        wi = max(PADW, ((wi + PADW - 1) // PADW) * PADW)
        ws.append(wi)
    kts = [(w + 127) // 128 for w in ws]
    offs = np.concatenate([[0], np.cumsum(ws)]).astype(int)
    kb = np.concatenate([[0], np.cumsum(kts)]).astype(int)
    return dict(
        a=a, slots=slots, ws=tuple(ws), kts=tuple(kts),
        offs=tuple(int(x) for x in offs[:-1]), T=int(offs[-1]),
        kb=tuple(int(x) for x in kb[:-1]), NKT=int(kb[-1]),
    )


# ---------------------------------------------------------------- program
_PROG_CACHE = {}


def _build_program(key):
    (T, ws, has_vbias) = key
    kts = tuple((w + 127) // 128 for w in ws)
    offs, kb = [], []
    o = k = 0
    for w, kt in zip(ws, kts):
        offs.append(o); kb.append(k); o += w; k += kt
    NKT = k
    BB = 8 + NKT + 1  # b_qk[0:4] b_oo[4:8] maskb[8:8+NKT] b_v[BB-1]
    dtb, dtf = mybir.dt.bfloat16, mybir.dt.float32
    cgroups = [(0, 1), (2, 3), (4, 5), (6, 7)]

    nc = bacc.Bacc("TRN2", target_bir_lowering=False, debug=False,
                   enable_asserts=False, num_devices=NCORES)

    def din(name, shape, dt):
        return nc.dram_tensor(name, shape, dt, kind="ExternalInput").ap()

    xT_d = din("xT", [258, T], dtb)
    wb_d = din("wb", [128, WBC], dtb)
    wr_d = din("wr", [2, 256], dtb)
    bb_d = din("bb", [128, BB], dtf)
    m01_d = din("mask01", [1, T], dtf)
    wvb_d = din("w_vb", [1, 256], dtb) if has_vbias else None
    out_d = nc.dram_tensor("val_out", [1, SPC], dtf, kind="ExternalOutput").ap()

    spl = min(T, 512)  # first xT column split (covers phase-A chunk 0)

    with tile.TileContext(nc) as tc:
        with (
            tc.tile_pool(name="const", bufs=1) as cp,
            tc.tile_pool(name="big", bufs=1) as bp,
            tc.tile_pool(name="vp", bufs=NKT) as vp,
            tc.tile_pool(name="ep", bufs=4) as ep,
            tc.tile_pool(name="rp", bufs=4) as rp,
            tc.tile_pool(name="pmm", bufs=2, space="PSUM") as pmm,
            tc.tile_pool(name="psc", bufs=2, space="PSUM") as psc,
            tc.tile_pool(name="pd", bufs=1, space="PSUM") as pd,
            tc.tile_pool(name="pc", bufs=1, space="PSUM") as pc,
        ):
            # ---- SBUF tiles
            wb_sb = cp.tile([128, WBC], dtb, tag="wb", name="wb")
            wr_sb = cp.tile([2, 256], dtb, tag="wr", name="wr")
            bb_sb = cp.tile([128, BB], dtf, tag="bb", name="bb")
            m01_sb = cp.tile([1, T], dtf, tag="m01", name="m01")
            ones_sb = cp.tile([128, 512], dtb, tag="ones", name="ones")
            dscr = cp.tile([1, 2], dtf, tag="dscr", name="dscr")
            xT_sb = [bp.tile([128, T], dtb, tag="xT0", name="xT0"),
                     bp.tile([128, T], dtb, tag="xT1", name="xT1"),
                     bp.tile([2, T], dtb, tag="xT2", name="xT2")]
            hT_sb = [bp.tile([128, T], dtb, tag=f"hT{f}", name=f"hT{f}")
                     for f in range(2)]
            q4_sb = [bp.tile([128, T], dtb, tag=f"q4{g}", name=f"q4{g}")
                     for g in range(2)]
            k4_sb = [bp.tile([128, T], dtb, tag=f"k4{g}", name=f"k4{g}")
                     for g in range(2)]
            # zero-padded per-head q: slot p holds head 4g+p's rows in band
            # 32p, zeros elsewhere -> K=128 score matmuls pick out one head
            # while sharing the k4 stationary (no base-0 relayout needed).
            qz_sb = [bp.tile([128, 4, T], dtb, tag=f"qz{g}", name=f"qz{g}")
                     for g in range(2)]
            ctxT_sb = [bp.tile([128, T], dtb, tag=f"cx{f}", name=f"cx{f}")
                       for f in range(2)]
            rsap_sb = [bp.tile([128, T], dtb, tag=f"rp{f}", name=f"rp{f}")
                       for f in range(2)]
            rsa_sb = [bp.tile([128, T], dtb, tag=f"rs{f}", name=f"rs{f}")
                      for f in range(2)]
            val_sb = bp.tile([1, T], dtf, tag="val", name="val")
            vscr_sb = bp.tile([1, 512], dtf, tag="vscr", name="vscr")
            out_sb = bp.tile([1, SPC], dtf, tag="out", name="out")
            if has_vbias:
                ones1_sb = cp.tile([1, T], dtb, tag="ones1", name="ones1")
                wvb_sb = cp.tile([1, 256], dtb, tag="wvb", name="wvb")

            # ---- DMA triggers (HWDGE only; order = need order)
            nc.scalar.dma_start(out=wb_sb[:, 0:WQK0], in_=wb_d[:, 0:WQK0])
            nc.scalar.dma_start(out=wb_sb[:, WQK0:WV0], in_=wb_d[:, WQK0:WV0])
            nc.gpsimd.dma_start(out=wr_sb, in_=wr_d)
            nc.gpsimd.dma_start(out=xT_sb[2], in_=xT_d[256:258, :])
            nc.sync.dma_start(out=xT_sb[0][:, 0:spl], in_=xT_d[0:128, 0:spl])
            nc.sync.dma_start(out=xT_sb[1][:, 0:spl], in_=xT_d[128:256, 0:spl])
            nc.sync.dma_start(out=bb_sb, in_=bb_d)
            if spl < T:
                nc.sync.dma_start(out=xT_sb[0][:, spl:T], in_=xT_d[0:128, spl:T])
                nc.sync.dma_start(out=xT_sb[1][:, spl:T], in_=xT_d[128:256, spl:T])
            nc.sync.dma_start(out=wb_sb[:, WV0:WBC], in_=wb_d[:, WV0:WBC])
            nc.sync.dma_start(out=m01_sb, in_=m01_d)
            if has_vbias:
                nc.sync.dma_start(out=wvb_sb, in_=wvb_d)
                nc.vector.memset(ones1_sb, 1.0)

            # exp table prefetch + constants
            nc.vector.memset(ones_sb, 1.0)
            nc.vector.memset(dscr[0:1, 0:1], 0.0)
            nc.scalar.activation(dscr[0:1, 1:2], dscr[0:1, 0:1], AF.Exp)

            mm = nc.tensor.matmul

            # PE warm-up: dense dummy matmuls on the ones tile while the
            # input DMAs stream, so HAM un-throttles before real work starts.
            wps = pmm.tile([128, 512], dtf, tag="mm", name="mm")
            for wi in range(6):
                mm(wps[0:64, 0:512], ones_sb[:, 0:64], ones_sb[:, 0:512],
                   start=True, stop=True)

            # ---------------- emission helpers
            def A_chunk(c0):
                cw = min(512, T - c0)
                cs = slice(c0, c0 + cw)
                for ft in range(2):
                    fs = slice(128 * ft, 128 * ft + 128)
                    hps = pmm.tile([128, 512], dtf, tag="mm", name="mm")
                    mm(hps[:, :cw], wb_sb[:, WIN0 + 128 * ft:WIN0 + 128 * ft + 128],
                       xT_sb[0][:, cs], start=True, stop=False)
                    mm(hps[:, :cw], wb_sb[:, WIN0 + 256 + 128 * ft:WIN0 + 384 + 128 * ft],
                       xT_sb[1][:, cs], start=False, stop=False)
                    mm(hps[:, :cw], wr_sb[:, fs], xT_sb[2][:, cs],
                       start=False, stop=True)
                    nc.scalar.activation(hT_sb[ft][:, cs], hps[:, :cw], AF.Relu)
                for m in range(4):
                    qps = pmm.tile([128, 512], dtf, tag="mm", name="mm")
                    mm(qps[:, :cw], wb_sb[:, WQK0 + 128 * m:WQK0 + 128 * m + 128],
                       hT_sb[0][:, cs], start=True, stop=False)
                    mm(qps[:, :cw], wb_sb[:, WQK0 + 512 + 128 * m:WQK0 + 640 + 128 * m],
                       hT_sb[1][:, cs], start=False, stop=True)
                    dst = (q4_sb if m < 2 else k4_sb)[m % 2]
                    nc.vector.tensor_scalar_add(dst[:, cs], qps[:, :cw],
                                                bb_sb[:, m:m + 1])

            vts = {}  # (slot, jj) -> bf16 v tile [nkz, 256]

            def V_slot(i):
                w, off = ws[i], offs[i]
                for jj in range(kts[i]):
                    nkz = min(128, w - 128 * jj)
                    t0 = off + 128 * jj
                    vps = pmm.tile([128, 256], dtf, tag="mm", name="mm")
                    mm(vps[0:nkz, :], hT_sb[0][:, t0:t0 + nkz],
                       wb_sb[:, WV0:WV0 + 256], start=True, stop=False)
                    mm(vps[0:nkz, :], hT_sb[1][:, t0:t0 + nkz],
                       wb_sb[:, WV0 + 256:WV0 + 512], start=False,
                       stop=not has_vbias)
                    if has_vbias:
                        mm(vps[0:nkz, :], ones1_sb[0:1, t0:t0 + nkz], wvb_sb,
                           start=False, stop=True)
                    vt = vp.tile([128, 256], dtb, tag="v", name="v")
                    nc.vector.tensor_copy(vt[0:nkz, :], vps[0:nkz, :])
                    vts[(i, jj)] = vt

            exps = {}  # (slot, jj) -> exp tile [nkz, 8, w]

            def S_slot(i):
                # scores via K=128 matmuls: stationary k4[g] (all 4 heads),
                # moving qz[g] slot p (head 4g+p's rows, zeros elsewhere).
                w, off = ws[i], offs[i]
                for jj in range(kts[i]):
                    nkz = min(128, w - 128 * jj)
                    t0 = off + 128 * jj
                    ti = kb[i] + jj
                    exp_t = ep.tile([128, 8, w], dtb, tag="exp", name="exp",
                                    padded_shape=[128, 8, 256])
                    if w > 128:
                        for g in range(2):
                            sp = psc.tile([128, 4, w], dtf, tag="sc", name="sc",
                                          padded_shape=[128, 4, 256])
                            for pp in range(0, 4, 2):
                                mm(sp[0:nkz, pp:pp + 2, 0:w],
                                   k4_sb[g][:, t0:t0 + nkz],
                                   qz_sb[g][:, pp:pp + 2, off:off + w],
                                   start=True, stop=True)
                            nc.scalar.activation(
                                exp_t[0:nkz, 4 * g:4 * g + 4, 0:w],
                                sp[0:nkz, :, 0:w], AF.Exp,
                                bias=bb_sb[0:nkz, 8 + ti:9 + ti], scale=SCALE)
                    else:
                        sp = psc.tile([128, 8, w], dtf, tag="sc", name="sc",
                                      padded_shape=[128, 8, 128])
                        nh = 2 if w > 64 else 4
                        for g in range(2):
                            for pp in range(0, 4, nh):
                                mm(sp[0:nkz, 4 * g + pp:4 * g + pp + nh, 0:w],
                                   k4_sb[g][:, t0:t0 + nkz],
                                   qz_sb[g][:, pp:pp + nh, off:off + w],
                                   start=True, stop=True)
                        nc.scalar.activation(
                            exp_t[0:nkz, :, 0:w], sp[0:nkz, :, 0:w], AF.Exp,
                            bias=bb_sb[0:nkz, 8 + ti:9 + ti], scale=SCALE)
                    exps[(i, jj)] = exp_t

            def D_slot(i):
                # den/ctx: per-band accumulation chains across k-tiles.  Each
                # 32-partition band holds an independent chain (per-element
                # has_written bits make band-disjoint chains in one bank safe);
                # the 4 bands' matmuls run concurrently via col tiling.
                w, kt, off = ws[i], kts[i], offs[i]
                dps = pd.tile([128, 2 * w], dtf, tag="dps", name="dps",
                              padded_shape=[128, 512])
                cps = pc.tile([128, 2 * w], dtf, tag="cps", name="cps",
                              padded_shape=[128, 512])
                for jj in range(kt):
                    nkz = min(128, w - 128 * jj)
                    exp_t = exps.pop((i, jj))
                    vt = vts.pop((i, jj))
                    first, last = jj == 0, jj == kt - 1
                    for j in range(4):
                        rh = exp_t[0:nkz, j:j + 5:4, 0:w]
                        mm(dps[32 * j:32 * j + 32, 0:2 * w],
                           ones_sb[0:nkz, 0:32], rh, start=first, stop=last,
                           tile_position=(0, 32 * j))
                    for g in range(2):
                        for j in range(4):
                            h = 4 * g + j
                            mm(cps[32 * j:32 * j + 32, g * w:(g + 1) * w],
                               vt[0:nkz, 32 * h:32 * h + 32],
                               exp_t[0:nkz, h, 0:w],
                               start=first and g == 0,
                               stop=last and g == 1,
                               tile_position=(0, 32 * j))
                rc = rp.tile([128, 2 * w], dtf, tag="rc", name="rc",
                             padded_shape=[128, 512])
                nc.vector.reciprocal_approx_fast(rc[:, 0:2 * w],
                                                 dps[:, 0:2 * w])
                for g in range(2):
                    nc.vector.tensor_mul(ctxT_sb[g][:, off:off + w],
                                         cps[:, g * w:(g + 1) * w],
                                         rc[:, g * w:(g + 1) * w])

            def C_group(gi):
                a, b = cgroups[gi]
                goff, gw = offs[a], ws[a] + ws[b]
                gs = slice(goff, goff + gw)
                for ft in range(2):
                    aps = pmm.tile([128, 512], dtf, tag="mm", name="mm")
                    mm(aps[:, :gw], wb_sb[:, WO0 + 128 * ft:WO0 + 128 * ft + 128],
                       ctxT_sb[0][:, gs], start=True, stop=False)
                    mm(aps[:, :gw], wb_sb[:, WO0 + 256 + 128 * ft:WO0 + 384 + 128 * ft],
                       ctxT_sb[1][:, gs], start=False, stop=True)
                    nc.vector.scalar_tensor_tensor(
                        rsap_sb[ft][:, gs], aps[:, :gw], bb_sb[:, 4 + ft:5 + ft],
                        hT_sb[ft][:, gs], OP.add, OP.add)
                for ft in range(2):
                    rps = pmm.tile([128, 512], dtf, tag="mm", name="mm")
                    mm(rps[:, :gw], wb_sb[:, WU0 + 128 * ft:WU0 + 128 * ft + 128],
                       rsap_sb[0][:, gs], start=True, stop=False)
                    mm(rps[:, :gw], wb_sb[:, WU0 + 256 + 128 * ft:WU0 + 384 + 128 * ft],
                       rsap_sb[1][:, gs], start=False, stop=True)
                    nc.scalar.activation(rsa_sb[ft][:, gs], rps[:, :gw],
                                         AF.Relu, bias=bb_sb[:, 6 + ft:7 + ft])
                vps = pmm.tile([1, 512], dtf, tag="mm", name="mm")
                mm(vps[0:1, :gw], wb_sb[:, WF0:WF0 + 1], rsa_sb[0][:, gs],
                   start=True, stop=False)
                mm(vps[0:1, :gw], wb_sb[:, WF0 + 1:WF0 + 2], rsa_sb[1][:, gs],
                   start=False, stop=True)
                vtmp = rp.tile([1, 512], dtf, tag="vt", name="vt")
                nc.vector.tensor_scalar_add(vtmp[0:1, 0:gw], vps[0:1, 0:gw],
                                            bb_sb[0:1, BB - 1:BB])
                nc.vector.scalar_tensor_tensor(
                    val_sb[0:1, gs], vtmp[0:1, 0:gw], 0.01, vtmp[0:1, 0:gw],
                    OP.mult, OP.max)
                for s in (a, b):
                    nc.vector.scalar_tensor_tensor(
                        vscr_sb[0:1, 0:ws[s]], val_sb[0:1, offs[s]:offs[s] + ws[s]],
                        1.0, m01_sb[0:1, offs[s]:offs[s] + ws[s]],
                        OP.mult, OP.mult, accum_out=out_sb[0:1, s:s + 1])

            def QZ_seg(c0, c1, engs):
                # band-slot copies q4 -> qz (same partitions, col shift);
                # g=0 and g=1 go to different trigger queues in parallel
                for g in range(2):
                    for p in range(4):
                        engs[g].dma_start(
                            out=qz_sb[g][32 * p:32 * p + 32, p, c0:c1],
                            in_=q4_sb[g][32 * p:32 * p + 32, c0:c1])

            # ---------------- emission schedule (priority order; the tile
            # scheduler dispatches by readiness, preferring earlier emission)
            spl1 = min(T, offs[2] if offs[2] > 0 else T)
            seg_bounds = sorted(set(min(x, T) for x in (offs[2], offs[5], T)))
            for g in range(2):
                for p in range(4):
                    eng = nc.gpsimd if (p % 2 == 0) else nc.vector
                    eng.memset(qz_sb[g][:, p, 0:spl1], 0.0)
            for g in range(2):
                for p in range(4):
                    if spl1 < T:
                        eng = nc.gpsimd if (p % 2 == 0) else nc.vector
                        eng.memset(qz_sb[g][:, p, spl1:T], 0.0)
            chunk_starts = list(range(0, T, 512))
            seg_prev = 0
            segq = []
            for se in seg_bounds:
                if se > seg_prev:
                    segq.append((seg_prev, se))
                    seg_prev = se
            seg_done = [False] * len(segq)
            seg_engs = [(nc.sync, nc.sync), (nc.sync, nc.sync),
                        (nc.sync, nc.sync)]

            def emit_segs(cols_done):
                for k, (a, b) in enumerate(segq):
                    if not seg_done[k] and b <= cols_done:
                        QZ_seg(a, b, seg_engs[min(k, 2)])
                        seg_done[k] = True

            A_chunk(chunk_starts[0])
            emit_segs(min(T, 512))
            V_slot(0); V_slot(1)
            S_slot(0)
            if len(chunk_starts) > 1:
                A_chunk(chunk_starts[1])
                emit_segs(min(T, 1024))
            D_slot(0)
            S_slot(1)
            for c0 in chunk_starts[2:]:
                A_chunk(c0)
            emit_segs(T)
            D_slot(1)
            V_slot(2); V_slot(3)
            S_slot(2)
            C_group(0)
            D_slot(2)
            S_slot(3); D_slot(3)
            V_slot(4); V_slot(5); V_slot(6); V_slot(7)
            C_group(1)
            S_slot(4); D_slot(4)
            S_slot(5); D_slot(5)
            C_group(2)
            S_slot(6); D_slot(6)
            S_slot(7); D_slot(7)
            C_group(3)
            nc.sync.dma_start(out=out_d, in_=out_sb)

    nc.compile()
    return nc


def get_program(plan, has_vbias):
    key = (plan["T"], plan["ws"], bool(has_vbias))
    if key not in _PROG_CACHE:
        _PROG_CACHE[key] = _build_program(key)
    return _PROG_CACHE[key]


# ---------------------------------------------------------------- host data
def _shared_inputs(W_in, b_in, W_qkv, b_qkv, W_o, b_o, W_out, b_out, W_v, b_v,
                   NKT):
    f32 = np.float32
    W_in = np.asarray(W_in, f32)
    b_qkv = np.asarray(b_qkv, f32)
    b_o, b_out = np.asarray(b_o, f32), np.asarray(b_out, f32)
    wb = np.zeros((128, WBC), f32)
    w_in_t = W_in[:, :256].T              # [256 in-feat, 256 out-feat]
    wb[:, WIN0:WIN0 + 256] = w_in_t[0:128]
    wb[:, WIN0 + 256:WIN0 + 512] = w_in_t[128:256]
    w_qk_t = np.asarray(W_qkv, f32)[:2 * E].T   # [256, 512]
    wb[:, WQK0:WQK0 + 512] = w_qk_t[0:128]
    wb[:, WQK0 + 512:WQK0 + 1024] = w_qk_t[128:256]
    w_v_t = np.asarray(W_qkv, f32)[2 * E:3 * E].T
    wb[:, WV0:WV0 + 256] = w_v_t[0:128]
    wb[:, WV0 + 256:WV0 + 512] = w_v_t[128:256]
    w_o_t = np.asarray(W_o, f32).T
    wb[:, WO0:WO0 + 256] = w_o_t[0:128]
    wb[:, WO0 + 256:WO0 + 512] = w_o_t[128:256]
    w_out_t = np.asarray(W_out, f32).T
    wb[:, WU0:WU0 + 256] = w_out_t[0:128]
    wb[:, WU0 + 256:WU0 + 512] = w_out_t[128:256]
    w_f_t = np.asarray(W_v, f32).T        # [256, 1]
    wb[:, WF0:WF0 + 1] = w_f_t[0:128]
    wb[:, WF0 + 1:WF0 + 2] = w_f_t[128:256]
    wr = np.stack([W_in[:, 256], np.asarray(b_in, f32)], axis=0)  # [2, 256]
    BB = 8 + NKT + 1
    bb = np.zeros((128, BB), np.float32)
    bb[:, 0:4] = b_qkv[:2 * E].reshape(4, 128).T
    bb[:, 4] = b_o[:128]; bb[:, 5] = b_o[128:]
    bb[:, 6] = b_out[:128]; bb[:, 7] = b_out[128:]
    bb[:, BB - 1] = float(np.asarray(b_v, f32).reshape(-1)[0])
    shared = {"wb": wb.astype(BF16), "wr": wr.astype(BF16), "bb": bb}
    has_vbias = bool(np.any(b_qkv[2 * E:] != 0))
    if has_vbias:
        shared["w_vb"] = b_qkv[2 * E:].reshape(1, 256).astype(BF16)
    return shared, has_vbias


def _core_inputs(plan, c, encoded_obs, shared):
    f32 = np.float32
    T, ws, offs, kts, kb, NKT = (plan["T"], plan["ws"], plan["offs"],
                                 plan["kts"], plan["kb"], plan["NKT"])
    a = plan["a"]
    xT = np.zeros((258, T), f32)
    m01 = np.zeros((1, T), f32)
    bb = shared["bb"].copy()
    p = np.arange(128)
    for i, s in enumerate(plan["slots"][c]):
        ai, w, off = int(a[s]), ws[i], offs[i]
        xT[0:256, off:off + ai] = np.asarray(encoded_obs[s, :ai, :], f32).T
        xT[256, off:off + ai] = ai / N
        xT[257, off:off + w] = 1.0
        m01[0, off:off + ai] = 1.0
        for jj in range(kts[i]):
            tok = 128 * jj + p
            bb[tok >= ai, 8 + kb[i] + jj] = NEG
    im = {"xT": xT.astype(BF16), "bb": bb, "mask01": m01}
    im["wb"] = shared["wb"]
    im["wr"] = shared["wr"]
    if "w_vb" in shared:
        im["w_vb"] = shared["w_vb"]
    return im


# ---------------------------------------------------------------- entry
def kernel(**inputs):
    global LAST_RESULT
    encoded_obs = np.asarray(inputs["encoded_obs"])
    actives = np.asarray(inputs["actives"]).reshape(-1)
    plan = _plan(actives)
    shared, has_vbias = _shared_inputs(
        inputs["W_in"], inputs["b_in"], inputs["W_qkv"], inputs["b_qkv"],
        inputs["W_o"], inputs["b_o"], inputs["W_out"], inputs["b_out"],
        inputs["W_v"], inputs["b_v"], plan["NKT"])
    nc = get_program(plan, has_vbias)
    in_maps = [_core_inputs(plan, c, encoded_obs, shared)
               for c in range(NCORES)]
    trace = bool(int(os.environ.get("KERNEL_TRACE", "0")))
    res = run_bass_kernel_spmd(nc, in_maps, core_ids=list(range(NCORES)),
                               trace=trace)
    LAST_RESULT = res
    out = np.zeros((B, 1), np.float32)
    for c in range(NCORES):
        vals = res.results[c]["val_out"].reshape(-1)
        for i, s in enumerate(plan["slots"][c]):
            out[s, 0] = vals[i]
    return out
